# revision 1
# baseline (speedup 1.0000x reference)
"""Trainium2 Bass kernel for a pre-LN transformer block (dense_transformer).

Reference computation (fp32, per batch element):
    x = x + Attn(LN1(x));  x = x + MLP(LN2(x))
with 16-head causal ALiBi attention (S=2048, D=2048) and a 4*D GELU MLP.

Distribution: 4 batches x 2-way head/tensor parallel = 8 cores.
Core c handles batch c//2 with pair-rank r=c%2:
  - attention: 8 local heads (r*8..r*8+7), all 2048 query positions. Scores
    are computed transposed [j(key) x i(query)] so the ALiBi +s_h*j term rides
    the ACT-exp per-partition bias and the -s_h*i term is a K=3 seed matmul
    (bf16 hi/lo/lolo rows reconstruct the fp32 column bias in PSUM). The
    combined exp argument qk/sqrt(hd) + s_h*(j-i) is <= O(1), so no running
    max is needed; per-query softmax scale factors cancel in the normalize.
  - softmax denominators via ones-lhsT matmuls accumulated alongside AV;
    normalization is fused into the AV PSUM->SBUF copy using a K=1 broadcast
    matmul of the reciprocal row.
  - a pair AllGather (split in two by head group so the first half overlaps
    the second head group's attention compute) swaps attention halves so each
    core owns 1024 tokens for the output projection, residual, LN2 and MLP
    (full 4*D hidden).
All per-core variation (weight slices, ALiBi slopes, token offsets) is input
DATA; the instruction stream is identical on all 8 cores (SPMD).

v2 structural changes vs v1 (same math):
  - q/k/v all projected up-front into SBUF; h_fm released before attention,
    which lets the Wo tile (first half) prefetch during the attention phase.
  - attention loop is head-outer so the first 4 heads' exchange (AllGather 1)
    overlaps the remaining heads' compute.
  - per-head seed rows loaded in one DMA each instead of per-(ct,h) slivers.
  - w2 double-buffered so its 8 MiB chunks stream under the down-proj.
  - constants packed into two shared tiles (every tile pads to 4 KiB; the 13
    separate const tiles wasted ~40 KiB of SBUF).

The walrus in this container encodes at most ONE sync wait per instruction,
so _legalize_waits() splits every multi-wait instruction into wait-only
EventSemaphore instructions inserted immediately before it on the same
engine stream - order-preserving, so semantics are unchanged. CoreSim
cannot execute wait-only EventSemaphore instructions, so numerical
validation (simcheck.py) runs on the pre-legalization build.
"""

import os
import sys

for _p in ("/opt/trn_rl_repo", "/opt/trn_rl_repo/concourse"):
    if os.path.isdir(_p) and _p not in sys.path:
        sys.path.append(_p)

import numpy as np
import ml_dtypes

import concourse.bass as bass
import concourse.mybir as mybir
import concourse.tile as tile
from contextlib import ExitStack

BF16 = mybir.dt.bfloat16
F32 = mybir.dt.float32
AF = mybir.ActivationFunctionType
ALU = mybir.AluOpType

REAL_CFG = dict(S=2048, D=2048, F=8192, H=16, GELU="Gelu")
LN_EPS = 1e-5
NEG = -1.0e6  # causal mask additive value (pre-exp)


def _cfg_derived(cfg):
    S, D, F, H = cfg["S"], cfg["D"], cfg["F"], cfg["H"]
    d = dict(cfg)
    d["HL"] = H // 2              # local heads per core
    d["HLW"] = d["HL"] * 128      # local head width (vd)
    d["DT"] = D // 128
    d["ST"] = S // 128
    d["QW"] = 512                 # q-chunk width (asserted below)
    d["CQ"] = S // 512
    d["OWN"] = S // 2
    d["OTT"] = d["OWN"] // 128
    d["FT"] = F // 128
    d["VDT"] = H
    d["HG"] = d["HL"] // 2        # heads per exchange group (2 groups)
    assert S % 512 == 0 and D % 512 == 0 and F % 512 == 0
    return d


# ------------------------------------------------------------ host prep ---


def _bf(x):
    return np.asarray(x, np.float32).astype(ml_dtypes.bfloat16)


def _split3(v):
    """Split fp32 array (last axis vectors) into 3 bf16 rows summing to it."""
    v = np.asarray(v, np.float32)
    r0 = v.astype(ml_dtypes.bfloat16)
    rem = v - r0.astype(np.float32)
    r1 = rem.astype(ml_dtypes.bfloat16)
    r2 = (rem - r1.astype(np.float32)).astype(ml_dtypes.bfloat16)
    return np.stack([r0, r1, r2])


def make_core_inputs(cfg, inputs, core):
    c = _cfg_derived(cfg)
    S, D, F, H, HL, ST = c["S"], c["D"], c["F"], c["H"], c["HL"], c["ST"]
    HLW, OWN, FT = c["HLW"], c["OWN"], c["FT"]
    b, r = core // 2, core % 2
    hd = 128
    f32 = np.float32

    x = np.asarray(inputs["x"][b], f32)
    g1 = np.asarray(inputs["ln1_w"], f32)
    c1 = np.asarray(inputs["ln1_b"], f32)
    g2 = np.asarray(inputs["ln2_w"], f32)
    c2 = np.asarray(inputs["ln2_b"], f32)
    Wqkv = np.asarray(inputs["Wqkv"], f32)
    bqkv = np.asarray(inputs["bqkv"], f32)
    Wo = np.asarray(inputs["Wo"], f32)
    bo = np.asarray(inputs["bo"], f32)
    W1 = np.asarray(inputs["W1"], f32)
    b1 = np.asarray(inputs["b1"], f32)
    W2 = np.asarray(inputs["W2"], f32)
    b2 = np.asarray(inputs["b2"], f32)
    slopes = np.asarray(inputs["slopes"], f32)

    Wq, Wk, Wv = Wqkv[:D], Wqkv[D:2 * D], Wqkv[2 * D:]
    bq, bk, bv = bqkv[:D], bqkv[D:2 * D], bqkv[2 * D:]

    lo, hi = r * HLW, (r + 1) * HLW
    sc = 1.0 / np.sqrt(hd)

    Wq_l = Wq[lo:hi] * g1[None, :]
    Wk_l = Wk[lo:hi] * g1[None, :]
    Wv_l = Wv[lo:hi] * g1[None, :]
    qb = (Wq[lo:hi] @ c1 + bq[lo:hi]) * sc
    kb = Wk[lo:hi] @ c1 + bk[lo:hi]
    cbo = Wo @ (Wv @ c1 + bv) + bo          # v-bias + bo folded constant [D]

    W1p = W1 * g2[None, :]
    b1p = W1 @ c2 + b1

    heads = np.arange(r * HL, r * HL + HL)
    sl = slopes[heads]
    jpos = np.arange(S, dtype=f32)

    expbias = np.zeros((128, HL * ST), f32)
    for h in range(HL):
        for jt in range(ST):
            expbias[:, h * ST + jt] = sl[h] * (jt * 128 + jpos[:128])

    seed = np.zeros((3, HL * S), f32)
    for h in range(HL):
        seed[:, h * S:(h + 1) * S] = _split3(-sl[h] * jpos)

    masks = np.zeros((128, 4 * 512), f32)
    jj = np.arange(128)[:, None]
    ii = np.arange(512)[None, :]
    for m in range(4):
        masks[:, m * 512:(m + 1) * 512] = np.where(m * 128 + jj <= ii, 0.0, NEG)

    return {
        "x_full": np.ascontiguousarray(x),
        "x_own": np.ascontiguousarray(x[r * OWN:(r + 1) * OWN]),
        "wq_t": np.ascontiguousarray(_bf(Wq_l.T * sc)),
        "wk_t": np.ascontiguousarray(_bf(Wk_l.T)),
        "wv_t": np.ascontiguousarray(_bf(Wv_l.T)),
        "wo_t": np.ascontiguousarray(_bf(Wo.T)),
        "w1_t": np.ascontiguousarray(_bf(W1p.T)),
        "w2_t": np.ascontiguousarray(_bf(W2.T)),
        "qb": np.ascontiguousarray(qb.reshape(HL, 128).T),
        "kb": np.ascontiguousarray(kb.reshape(HL, 128).T),
        "b1c": np.ascontiguousarray(b1p.reshape(FT, 128).T),
        "obias": np.ascontiguousarray(_split3(cbo)[:2]),
        "b2row": np.ascontiguousarray(_split3(b2)[:2]),
        "expbias": expbias,
        "seed": np.ascontiguousarray(seed.astype(ml_dtypes.bfloat16)),
        "masks": np.ascontiguousarray(masks.astype(ml_dtypes.bfloat16)),
        "ident": np.eye(128, dtype=f32).astype(ml_dtypes.bfloat16),
        "sel": np.ascontiguousarray(
            np.repeat((np.arange(2) == r).astype(f32)[None, :], 128, axis=0)),
    }


# ------------------------------------------------------------ the kernel ---


def build_kernel(cfg, legalize=True):
    c = _cfg_derived(cfg)
    S, D, F = c["S"], c["D"], c["F"]
    HL, HLW, DT, ST = c["HL"], c["HLW"], c["DT"], c["ST"]
    CQ, QW, OWN, OTT, FT, VDT, HG = (c["CQ"], c["QW"], c["OWN"], c["OTT"],
                                     c["FT"], c["VDT"], c["HG"])
    GELU = getattr(AF, cfg["GELU"])

    # Single SWDGE sem lane: every DMA rides one FIFO queue (qPoolDynamic)
    # anyway, but Tile's default 8-lane round-robin sem assignment makes
    # slot-reuse DMAs wait on several DMASW sems at once, and the DMA ISA
    # encodes at most 2 waits (walrus "Too many sync wait commands").
    import concourse.tile_sem_assignment as tsa
    tsa.NUM_SWDGE_GLOBAL_SEMS = 1

    nc = bass.Bass()

    def param(name, shape, dt):
        return nc.declare_dram_parameter(name, shape, dt, isOutput=False)

    x_full_d = param("x_full", [S, D], F32)
    x_own_d = param("x_own", [OWN, D], F32)
    wq_d = param("wq_t", [D, HLW], BF16)
    wk_d = param("wk_t", [D, HLW], BF16)
    wv_d = param("wv_t", [D, HLW], BF16)
    wo_d = param("wo_t", [D, D], BF16)
    w1_d = param("w1_t", [D, F], BF16)
    w2_d = param("w2_t", [F, D], BF16)
    qb_d = param("qb", [128, HL], F32)
    kb_d = param("kb", [128, HL], F32)
    b1c_d = param("b1c", [128, FT], F32)
    obias_d = param("obias", [2, D], BF16)
    b2row_d = param("b2row", [2, D], BF16)
    expb_d = param("expbias", [128, HL * ST], F32)
    seed_d = param("seed", [3, HL * S], BF16)
    masks_d = param("masks", [128, 4 * 512], BF16)
    ident_d = param("ident", [128, 128], BF16)
    sel_d = param("sel", [128, 2], F32)
    out_d = nc.declare_dram_parameter("out", [OWN, D], F32, isOutput=True)

    groups = [[0, 1], [2, 3], [4, 5], [6, 7]]
    # exchange chunk sizes (heads): small tail chunks so the last exchange
    # (the only exposed one) is cheap
    CHUNKS = [2, 2, 2, 1, 1] if HL == 8 else [1] * HL
    NG = len(CHUNKS)
    CH_OF = []                    # head -> chunk index
    for gi, n in enumerate(CHUNKS):
        CH_OF += [gi] * n
    CH_BASE = [sum(CHUNKS[:gi]) for gi in range(NG)]

    with tile.TileContext(nc) as tc, ExitStack() as top:
        def dma(out_ap, in_ap):
            nc.gpsimd.dma_start(out_ap, in_ap)

        def dma_blk(sbuf_ap, dram_ap):
            """DMA DRAM [T*128, N] into SBUF [128, T*N] (block t at t*N)."""
            rows = dram_ap.shape[0]
            t = rows // 128
            dma(sbuf_ap.rearrange("p (t f) -> p t f", t=t),
                dram_ap.rearrange("(t p) f -> p t f", p=128))

        dram = top.enter_context(tc.tile_pool(name="dram", bufs=1,
                                              space="DRAM"))
        exch_g = [dram.tile([2, CHUNKS[i] * 128, OWN], BF16,
                            name=f"exch{i}", tag=f"exch{i}")
                  for i in range(NG)]
        ago_g = [dram.tile([2, 2, CHUNKS[i] * 128, OWN], BF16,
                           name=f"ago{i}", tag=f"ago{i}") for i in range(NG)]
        x2buf = dram.tile([OWN, D], F32)
        gbuf = dram.tile([F, OWN], BF16)

        es_a, es_qkv, es_wo, es_b = (ExitStack(), ExitStack(), ExitStack(),
                                     ExitStack())
        const = top.enter_context(tc.tile_pool(name="const", bufs=1))
        pool_a = es_a.enter_context(tc.tile_pool(name="slotA", bufs=1))

        # Packed constants: every tile pads to 4 KiB/partition, so the many
        # small tiles are packed into two wide ones and sliced by view.
        # bf16 pack: masks[0:2048] | ident[2048:2176] | ones[2176:2304]
        pk_b = const.tile([128, 4 * 512 + 128 + 128], BF16)
        masks = pk_b[:, 0:4 * 512]
        ident = pk_b[:, 4 * 512:4 * 512 + 128]
        _ones = pk_b[:, 4 * 512 + 128:4 * 512 + 256]
        ones3 = _ones[0:3, 0:128]
        ones2 = _ones[0:2, 0:128]
        onesc = _ones[:, 0:1]
        onesr = _ones[0:1, 0:128]
        # f32 pack: expb[0:HL*ST] | qb | kb | b1c | sel | epsc
        _c0 = HL * ST
        pk_f = const.tile([128, _c0 + 2 * HL + FT + 3], F32)
        expb = pk_f[:, 0:_c0]
        qb = pk_f[:, _c0:_c0 + HL]
        kb = pk_f[:, _c0 + HL:_c0 + 2 * HL]
        b1c = pk_f[:, _c0 + 2 * HL:_c0 + 2 * HL + FT]
        sel = pk_f[:, _c0 + 2 * HL + FT:_c0 + 2 * HL + FT + 2]
        epsc = pk_f[:, _c0 + 2 * HL + FT + 2:_c0 + 2 * HL + FT + 3]

        dma(ident, ident_d[:])
        nc.vector.memset(_ones, 1.0)
        nc.vector.memset(epsc, LN_EPS)
        dma(qb, qb_d[:])
        dma(kb, kb_d[:])

        # ---- LayerNorm helper (normalized output only; w/b pre-folded) ---
        LNG = D // 512 if D >= 512 else 1

        def layernorm_tile(stat, xt, out_bf):
            st = stat.tile([128, 6 * LNG + 4], F32, tag="lnstat")
            st6 = st[:, 0:6 * LNG]
            ag = st[:, 6 * LNG:6 * LNG + 2]
            sd = st[:, 6 * LNG + 2:6 * LNG + 3]
            rr = st[:, 6 * LNG + 3:6 * LNG + 4]
            for g in range(LNG):
                nc.vector.bn_stats(st6[:, 6 * g:6 * (g + 1)],
                                   xt[:, 512 * g:512 * (g + 1)])
            nc.vector.bn_aggr(ag, st6)
            nc.scalar.activation(sd, ag[:, 1:2], AF.Sqrt,
                                 bias=epsc, scale=1.0)
            nc.vector.reciprocal(rr, sd)
            nc.vector.tensor_scalar(
                out_bf[:], xt[:], scalar1=ag[:, 0:1], scalar2=rr,
                op0=ALU.subtract, op1=ALU.mult)

        # ---- phase A: LN1 + transpose into h_fm ---------------------------
        h_fm = pool_a.tile([128, DT * S], BF16, tag="a")
        with ExitStack() as ph:
            xpool = ph.enter_context(tc.tile_pool(name="ln_x", bufs=2))
            stat = ph.enter_context(tc.tile_pool(name="ln_stat", bufs=2))
            hbf = ph.enter_context(tc.tile_pool(name="ln_h", bufs=2))
            tps = ph.enter_context(
                tc.tile_pool(name="tps", bufs=6, space="PSUM"))
            for tt in range(ST):
                xt = xpool.tile([128, D], F32)
                dma(xt[:], x_full_d[tt * 128:(tt + 1) * 128, :])
                ht = hbf.tile([128, D], BF16)
                layernorm_tile(stat, xt, ht)
                for dt in range(DT):
                    tp = tps.tile([128, 128], BF16)
                    nc.tensor.transpose(
                        tp[:], ht[:, dt * 128:(dt + 1) * 128], ident)
                    nc.vector.tensor_copy(
                        h_fm[:, dt * S + tt * 128: dt * S + (tt + 1) * 128],
                        tp[:])

        # ---- phase B: Q, K and V projections ------------------------------
        pool_q = es_qkv.enter_context(tc.tile_pool(name="slotQ", bufs=1, side="right"))
        pool_k = es_qkv.enter_context(tc.tile_pool(name="slotK", bufs=1, side="right"))
        pool_v = es_qkv.enter_context(tc.tile_pool(name="slotV", bufs=1, side="right"))
        q_sb = pool_q.tile([128, HL * S], BF16, tag="q")
        k_sb = pool_k.tile([128, HL * S], BF16, tag="k")
        v_sb = pool_v.tile([128, ST * HLW], BF16, tag="v")
        with ExitStack() as ph:
            mps = ph.enter_context(
                tc.tile_pool(name="bps", bufs=2, space="PSUM"))
            wpool = ph.enter_context(tc.tile_pool(name="slotW", bufs=1, side="right"))

            wk_sb = wpool.tile([128, DT * HLW], BF16, tag="w")
            dma_blk(wk_sb[:], wk_d.ap())
            for h in range(HL):
                for ch in range(CQ):
                    ps = mps.tile([128, QW], F32)
                    for dt in range(DT):
                        nc.tensor.matmul(
                            ps[:],
                            wk_sb[:, dt * HLW + h * 128:
                                  dt * HLW + (h + 1) * 128],
                            h_fm[:, dt * S + ch * QW: dt * S + (ch + 1) * QW],
                            start=(dt == 0), stop=(dt == DT - 1))
                    nc.vector.tensor_scalar_add(
                        k_sb[:, h * S + ch * QW: h * S + (ch + 1) * QW],
                        ps[:], kb[:, h:h + 1])

            wv_sb = wpool.tile([128, DT * HLW], BF16, tag="w")
            dma_blk(wv_sb[:], wv_d.ap())
            VCW = min(512, HLW)
            for jt in range(ST):
                for vc in range(HLW // VCW):
                    ps = mps.tile([128, VCW], F32)
                    for dt in range(DT):
                        nc.tensor.matmul(
                            ps[:],
                            h_fm[:, dt * S + jt * 128: dt * S + (jt + 1) * 128],
                            wv_sb[:, dt * HLW + vc * VCW:
                                  dt * HLW + (vc + 1) * VCW],
                            start=(dt == 0), stop=(dt == DT - 1))
                    nc.vector.tensor_copy(
                        v_sb[:, jt * HLW + vc * VCW:
                             jt * HLW + (vc + 1) * VCW],
                        ps[:])

            wq_sb = wpool.tile([128, DT * HLW], BF16, tag="w")
            dma_blk(wq_sb[:], wq_d.ap())
            for h in range(HL):
                for ch in range(CQ):
                    ps = mps.tile([128, QW], F32)
                    for dt in range(DT):
                        nc.tensor.matmul(
                            ps[:],
                            wq_sb[:, dt * HLW + h * 128:
                                  dt * HLW + (h + 1) * 128],
                            h_fm[:, dt * S + ch * QW: dt * S + (ch + 1) * QW],
                            start=(dt == 0), stop=(dt == DT - 1))
                    nc.vector.tensor_scalar_add(
                        q_sb[:, h * S + ch * QW: h * S + (ch + 1) * QW],
                        ps[:], qb[:, h:h + 1])

        es_a.close()   # h_fm released; wo_sb halves can land in its zone

        # ---- phase C: attention (head-outer) ------------------------------
        # Wo first half + obias prefetch: issued first so the 4 MiB load
        # streams while the attention loop runs (second half at phase E).
        pool_wo0 = es_wo.enter_context(tc.tile_pool(name="slotWo0", bufs=1))
        ob_p = es_wo.enter_context(tc.tile_pool(name="ob", bufs=1))
        HVD = (VDT // 2) * D
        wo_h0 = pool_wo0.tile([128, HVD], BF16, tag="wo0")
        dma_blk(wo_h0[:], wo_d[0:D // 2, :])
        obias = ob_p.tile([2, D], BF16)
        dma(obias[:], obias_d[:])
        dma(masks, masks_d[:])
        dma(expb, expb_d[:])
        dma(sel, sel_d[:])

        with ExitStack() as ph:
            seed_p = ph.enter_context(tc.tile_pool(name="seedp", bufs=2))
            att_p = ph.enter_context(tc.tile_pool(name="att", bufs=3))
            bcn_p = ph.enter_context(tc.tile_pool(name="bcn", bufs=2))
            den_p = ph.enter_context(tc.tile_pool(name="den", bufs=2))
            oat_p = ph.enter_context(tc.tile_pool(name="oat", bufs=2))
            ps_s = ph.enter_context(
                tc.tile_pool(name="pss", bufs=2, space="PSUM"))
            ps_a = ph.enter_context(
                tc.tile_pool(name="psa", bufs=2, space="PSUM"))
            ps_d = ph.enter_context(
                tc.tile_pool(name="psd", bufs=2, space="PSUM"))
            ps_b = ph.enter_context(
                tc.tile_pool(name="psb", bufs=1, space="PSUM"))
            for h in range(HL):
                seedt = seed_p.tile([3, S], BF16)
                dma(seedt[:], seed_d[:, h * S:(h + 1) * S])
                exch = exch_g[CH_OF[h]]
                hrow = (h - CH_BASE[CH_OF[h]]) * 128
                for ct in range(CQ):
                    njt = min(ST, (ct + 1) * (QW // 128))
                    pav = ps_a.tile([128, QW], F32)
                    pden = ps_d.tile([1, QW], F32)
                    for jt in range(njt):
                        pss = ps_s.tile([128, QW], F32)
                        nc.tensor.matmul(
                            pss[:], ones3,
                            seedt[:, ct * QW:(ct + 1) * QW],
                            start=True, stop=False)
                        nc.tensor.matmul(
                            pss[:],
                            k_sb[:, h * S + jt * 128: h * S + (jt + 1) * 128],
                            q_sb[:, h * S + ct * QW: h * S + (ct + 1) * QW],
                            start=False, stop=True)
                        m = jt - ct * (QW // 128)
                        if 0 <= m < 4:
                            nc.vector.tensor_add(
                                pss[:], pss[:],
                                masks[:, m * 512: m * 512 + QW])
                        at = att_p.tile([128, QW], BF16)
                        nc.scalar.activation(
                            at[:], pss[:], AF.Exp,
                            bias=expb[:, h * ST + jt: h * ST + jt + 1],
                            scale=1.0)
                        nc.tensor.matmul(
                            pav[:],
                            v_sb[:, jt * HLW + h * 128:
                                 jt * HLW + (h + 1) * 128],
                            at[:], start=(jt == 0), stop=(jt == njt - 1))
                        nc.tensor.matmul(
                            pden[:], onesc, at[:],
                            start=(jt == 0), stop=(jt == njt - 1))
                    den = den_p.tile([1, 2 * QW], F32, tag="denf")
                    dsb = den[:, 0:QW]
                    rec = den[:, QW:2 * QW]
                    nc.vector.tensor_copy(dsb, pden[:])
                    nc.vector.reciprocal(rec, dsb)
                    recb = den_p.tile([1, QW], BF16, tag="denb")
                    nc.vector.tensor_copy(recb[:], rec)
                    pbc = ps_b.tile([128, QW], F32)
                    nc.tensor.matmul(pbc[:], onesr, recb[:],
                                     start=True, stop=True)
                    bcn = bcn_p.tile([128, QW], F32)
                    nc.vector.tensor_copy(bcn[:], pbc[:])
                    oat = oat_p.tile([128, QW], BF16)
                    nc.vector.scalar_tensor_tensor(
                        oat[:], pav[:], 1.0, bcn[:],
                        op0=ALU.mult, op1=ALU.mult)
                    for half in range(2):
                        a = max(ct * QW, half * OWN)
                        bnd = min((ct + 1) * QW, (half + 1) * OWN)
                        if a < bnd:
                            dma(
                                exch[half, hrow:hrow + 128,
                                     a - half * OWN: bnd - half * OWN],
                                oat[:, a - ct * QW: bnd - ct * QW])
                # chunk done -> exchange it under the remaining heads'
                # compute (only the last chunk's exchange is exposed)
                if h == HL - 1 or CH_OF[h + 1] != CH_OF[h]:
                    g = CH_OF[h]
                    nc.gpsimd.collective_compute(
                        "AllGather", ALU.bypass, replica_groups=groups,
                        ins=[exch_g[g].opt()], outs=[ago_g[g].opt()])

        es_qkv.close()

        # ---- phase E: out-proj + residual + LN2 + transpose --------------
        pool_wo1 = es_wo.enter_context(tc.tile_pool(name="slotWo1", bufs=1))
        wo_h1 = pool_wo1.tile([128, HVD], BF16, tag="wo1")
        dma_blk(wo_h1[:], wo_d[D // 2:D, :])

        w1_p = es_b.enter_context(tc.tile_pool(name="w1", bufs=4,
                                               side="right"))
        pool_b = es_b.enter_context(tc.tile_pool(name="slotB", bufs=1, side="right"))
        es_attg = ExitStack()
        pool_c = es_attg.enter_context(tc.tile_pool(name="slotC", bufs=1, side="right"))
        h2_fm = pool_b.tile([128, DT * OWN], BF16, tag="b")
        with ExitStack() as ph:
            agp = ph.enter_context(tc.tile_pool(name="agp", bufs=2))
            xo_p = ph.enter_context(tc.tile_pool(name="xo", bufs=2))
            x2_p = ph.enter_context(tc.tile_pool(name="x2", bufs=2))
            h2_p = ph.enter_context(tc.tile_pool(name="h2", bufs=1))
            stat = ph.enter_context(tc.tile_pool(name="e_stat", bufs=2))
            ps_o = ph.enter_context(
                tc.tile_pool(name="pso", bufs=2, space="PSUM"))
            tps = ph.enter_context(
                tc.tile_pool(name="etps", bufs=6, space="PSUM"))

            attg = pool_c.tile([128, VDT * OWN], BF16, tag="c")
            for g in range(NG):
                for s in range(2):
                    for hh in range(CHUNKS[g]):
                        h = CH_BASE[g] + hh
                        g0 = agp.tile([128, OWN], BF16, tag="g")
                        dma(g0[:], ago_g[g][s, 0, hh * 128:(hh + 1) * 128, :])
                        g1 = agp.tile([128, OWN], BF16, tag="g")
                        dma(g1[:], ago_g[g][s, 1, hh * 128:(hh + 1) * 128, :])
                        dst = attg[:, (s * HL + h) * OWN:
                                   (s * HL + h + 1) * OWN]
                        nc.vector.tensor_scalar_mul(dst, g0[:], sel[:, 0:1])
                        nc.vector.scalar_tensor_tensor(
                            dst, g1[:], sel[:, 1:2], dst,
                            op0=ALU.mult, op1=ALU.add)
            for it in range(OTT):
                x2 = x2_p.tile([128, D], F32)
                for dc in range(D // 512):
                    po = ps_o.tile([128, 512], F32)
                    nc.tensor.matmul(
                        po[:], ones2, obias[:, dc * 512:(dc + 1) * 512],
                        start=True, stop=False)
                    for v in range(VDT):
                        wo_sb = wo_h0 if v < VDT // 2 else wo_h1
                        vv = v if v < VDT // 2 else v - VDT // 2
                        nc.tensor.matmul(
                            po[:],
                            attg[:, v * OWN + it * 128:
                                 v * OWN + (it + 1) * 128],
                            wo_sb[:, vv * D + dc * 512: vv * D + (dc + 1) * 512],
                            start=False, stop=(v == VDT - 1))
                    xo = xo_p.tile([128, 512], F32)
                    dma(
                        xo[:],
                        x_own_d[it * 128:(it + 1) * 128,
                                dc * 512:(dc + 1) * 512])
                    nc.vector.tensor_add(
                        x2[:, dc * 512:(dc + 1) * 512], po[:], xo[:])
                dma(x2buf[it * 128:(it + 1) * 128, :], x2[:])
                h2 = h2_p.tile([128, D], BF16)
                layernorm_tile(stat, x2, h2)
                for dt in range(DT):
                    tp = tps.tile([128, 128], BF16)
                    nc.tensor.transpose(
                        tp[:], h2[:, dt * 128:(dt + 1) * 128], ident)
                    nc.vector.tensor_copy(
                        h2_fm[:, dt * OWN + it * 128:
                              dt * OWN + (it + 1) * 128],
                        tp[:])

        es_attg.close()
        es_wo.close()

        # ---- phase F1: MLP up-proj + GELU -> gbuf ------------------------
        w2_p = top.enter_context(tc.tile_pool(name="w2", bufs=2))
        with ExitStack() as ph:
            gst_p = ph.enter_context(tc.tile_pool(name="gst", bufs=3))
            ps_m = ph.enter_context(
                tc.tile_pool(name="psm", bufs=2, space="PSUM"))
            dma(b1c, b1c_d[:])
            W1C = min(512, OWN)
            for oc in range(OWN // W1C):
                for ft in range(FT):
                    w1t = w1_p.tile([128, DT * 128], BF16)
                    dma_blk(w1t[:], w1_d[:, ft * 128:(ft + 1) * 128])
                    ps = ps_m.tile([128, W1C], F32)
                    for dt in range(DT):
                        nc.tensor.matmul(
                            ps[:],
                            w1t[:, dt * 128:(dt + 1) * 128],
                            h2_fm[:, dt * OWN + oc * W1C:
                                  dt * OWN + (oc + 1) * W1C],
                            start=(dt == 0), stop=(dt == DT - 1))
                    gt = gst_p.tile([128, W1C], BF16)
                    nc.scalar.activation(gt[:], ps[:], GELU,
                                         bias=b1c[:, ft:ft + 1], scale=1.0)
                    dma(
                        gbuf[ft * 128:(ft + 1) * 128,
                             oc * W1C:(oc + 1) * W1C], gt[:])

        es_b.close()

        # ---- phase F2: MLP down-proj + residual -> out -------------------
        with ExitStack() as ph:
            gs_p = ph.enter_context(tc.tile_pool(name="gs", bufs=2))
            b2_p = ph.enter_context(tc.tile_pool(name="b2", bufs=1))
            x2s_p = ph.enter_context(tc.tile_pool(name="x2s", bufs=3))
            o_p = ph.enter_context(tc.tile_pool(name="osb", bufs=3))
            ps_m = ph.enter_context(
                tc.tile_pool(name="psm2", bufs=2, space="PSUM"))
            b2row = b2_p.tile([2, D], BF16)
            dma(b2row[:], b2row_d[:])
            NDC = D // 512
            for dp in range(NDC // 2):
                w2a = w2_p.tile([128, FT * 512], BF16, name=f"w2a{dp}",
                                tag="w2")
                dma_blk(w2a[:], w2_d[:, (2 * dp) * 512:(2 * dp + 1) * 512])
                w2b = w2_p.tile([128, FT * 512], BF16, name=f"w2b{dp}",
                                tag="w2")
                dma_blk(w2b[:], w2_d[:, (2 * dp + 1) * 512:(2 * dp + 2) * 512])
                for it in range(OTT):
                    gs = gs_p.tile([128, FT * 128], BF16)
                    dma_blk(gs[:], gbuf[:, it * 128:(it + 1) * 128])
                    for half, w2t in ((0, w2a), (1, w2b)):
                        dc = 2 * dp + half
                        ps = ps_m.tile([128, 512], F32)
                        nc.tensor.matmul(
                            ps[:], ones2, b2row[:, dc * 512:(dc + 1) * 512],
                            start=True, stop=False)
                        for ft in range(FT):
                            nc.tensor.matmul(
                                ps[:],
                                gs[:, ft * 128:(ft + 1) * 128],
                                w2t[:, ft * 512:(ft + 1) * 512],
                                start=False, stop=(ft == FT - 1))
                        x2t = x2s_p.tile([128, 512], F32)
                        dma(
                            x2t[:],
                            x2buf[it * 128:(it + 1) * 128,
                                  dc * 512:(dc + 1) * 512])
                        ot = o_p.tile([128, 512], F32)
                        nc.vector.tensor_add(ot[:], ps[:], x2t[:])
                        dma(
                            out_d[it * 128:(it + 1) * 128,
                                  dc * 512:(dc + 1) * 512],
                            ot[:])

    if legalize:
        _legalize_waits(nc)
    return nc


def _legalize_waits(nc):
    """walrus on this container encodes at most ONE sync wait per DMA/branch
    instruction. Tile emits several (reader-WAR + DMA-lane WAW). Waits are
    executed by the issuing engine's sequencer in program order, so hoisting
    all-but-one wait onto wait-only EventSemaphore instructions inserted
    immediately before it on the same engine stream is semantics-preserving."""
    n_split = 0
    for fn in nc.m.functions:
        for bb in fn.blocks:
            out = []
            for inst in bb.instructions:
                si = inst.sync_info
                waits = list(si.on_wait) if si and si.on_wait else []
                if len(waits) > 1:
                    # merge same-sem waits to the max value
                    merged = {}
                    for w in waits:
                        k = (w.sync_type, w.id, w.wait_mode)
                        if k not in merged or merged[k].wait_value < w.wait_value:
                            merged[k] = w
                    waits = list(merged.values())
                    for w in waits[:-1]:
                        es = mybir.InstEventSemaphore(
                            name=f"{inst.name}-wsplit{n_split}",
                            engine=inst.engine,
                            ins=[], outs=[],
                            sync_info=mybir.SyncInfo(on_wait=[w], on_update=[]),
                        )
                        out.append(es)
                        n_split += 1
                    inst.sync_info = mybir.SyncInfo(
                        on_wait=[waits[-1]],
                        on_update=list(si.on_update) if si.on_update else [])
                out.append(inst)
            bb.instructions[:] = out


# ------------------------------------------------------------- the entry ---

_BUILT = {}


def _get_nc(cfg_key=None):
    if "nc" not in _BUILT:
        _BUILT["nc"] = build_kernel(REAL_CFG)
    return _BUILT["nc"]


def kernel(**inputs):
    cfg = REAL_CFG
    c = _cfg_derived(cfg)
    nc = _get_nc()
    in_maps = [make_core_inputs(cfg, inputs, core) for core in range(8)]
    from concourse.bass_utils import run_bass_kernel_spmd
    res = run_bass_kernel_spmd(nc, in_maps, list(range(8)))
    B = np.asarray(inputs["x"]).shape[0]
    S, D, OWN = cfg["S"], cfg["D"], c["OWN"]
    out = np.empty((B, S, D), np.float32)
    for core in range(8):
        b, r = core // 2, core % 2
        out[b, r * OWN:(r + 1) * OWN, :] = res.results[core]["out"]
    return out



# revision 2
# speedup vs baseline: 1.3957x; 1.3957x over previous
"""Trainium2 Bass kernel for a pre-LN transformer block (dense_transformer).

Reference computation (fp32, per batch element):
    x = x + Attn(LN1(x));  x = x + MLP(LN2(x))
with 16-head causal ALiBi attention (S=2048, D=2048) and a 4*D GELU MLP.

Distribution: 4 batches x 2-way head/tensor parallel = 8 cores.
Core c handles batch c//2 with pair-rank r=c%2:
  - attention: 8 local heads (r*8..r*8+7), all 2048 query positions. Scores
    are computed transposed [j(key) x i(query)]; the full ALiBi+softmax term
    rides the ACT-exp per-partition bias alone: the bias encodes
    s_h*(j - center_u) for a per-query sub-chunk center, and the implied
    per-query-column scale e^{s_h*(i-center_u)} is constant across key tiles
    so it cancels between the AV accumulation and the softmax denominator.
    No seed matmul is needed. Steep-slope head slots use 128-wide exp
    sub-chunks (fp32/bf16 range control); shallow slots use 256/512-wide
    ones (fewer ACT instructions). Head slot grain is compile-time; which
    head lives in a slot is per-core DATA, so the stream stays SPMD.
  - softmax denominators via ones-lhsT matmuls accumulated alongside AV;
    normalization is fused into the AV PSUM->SBUF copy using a K=1 broadcast
    matmul of the reciprocal row.
  - a pair AllGather (chunked by head group) swaps attention halves so each
    core owns 1024 tokens for the output projection, residual, LN2 and MLP;
    gathered chunks are assembled into the Wo operand DURING the remaining
    heads' attention compute (only the last 1-head chunk is exposed).
All per-core variation (weight slices, ALiBi slopes, token offsets) is input
DATA; the instruction stream is identical on all 8 cores (SPMD).

v3 structural changes vs v2 (same math):
  - fused MLP: GELU output g stays resident in SBUF in f-partition layout
    (exactly the lhsT layout the down-projection needs), processed in two
    512-token halves; the 48 MiB DRAM round-trip of v2 (gbuf write + narrow
    256B-line reload) is gone. w1 streams in 512-col chunks (1KB DMA lines,
    ~3x the measured single-queue bandwidth of 256B lines), w2 in 256-col
    chunks double-buffered.
  - two DMA queues: residual/x traffic and AllGather-result reads ride the
    SP engine's hardware-DGE queue; weight/exchange streams keep the gpsimd
    software-DGE queue. This removes FIFO head-of-line blocking between
    independent streams (all DMA previously serialized on one queue).
    (Do NOT issue DMAs from the ACT queue: their sync waits stall the ACT
    sequencer and delay GELU/exp work behind them - measured regression.)
  - x streamed in bf16 for the LN1 pass (residual path keeps fp32 x_own),
    halving the startup DMA and doubling DVE throughput there; transpose
    PSUM->SBUF copies alternate DVE/ACT.
  - scores PSUM pool depth 3 so the PE can run ahead of ACT exp.

Measured on 8xTRN2 (slope method, amortizing the ~85 ms axon round-trip and
~0.5 ms per-exec runtime overhead): v2 3.77 ms -> v3 2.22 ms per execution.

The walrus in this container encodes at most ONE sync wait per instruction,
so _legalize_waits() splits every multi-wait instruction into wait-only
EventSemaphore instructions inserted immediately before it on the same
engine stream - order-preserving, so semantics are unchanged.
"""

import os
import sys

for _p in ("/opt/trn_rl_repo", "/opt/trn_rl_repo/concourse"):
    if os.path.isdir(_p) and _p not in sys.path:
        sys.path.append(_p)

import numpy as np
import ml_dtypes

import concourse.bass as bass
import concourse.mybir as mybir
import concourse.tile as tile
from contextlib import ExitStack

BF16 = mybir.dt.bfloat16
F32 = mybir.dt.float32
AF = mybir.ActivationFunctionType
ALU = mybir.AluOpType

REAL_CFG = dict(S=2048, D=2048, F=8192, H=16, GELU="Gelu")
LN_EPS = 1e-5
NEG = -1.0e6  # causal mask additive value (pre-exp)


def _cfg_derived(cfg):
    S, D, F, H = cfg["S"], cfg["D"], cfg["F"], cfg["H"]
    d = dict(cfg)
    d["HL"] = H // 2              # local heads per core
    d["HLW"] = d["HL"] * 128      # local head width (vd)
    d["DT"] = D // 128
    d["ST"] = S // 128
    d["QW"] = 512                 # q-chunk width (asserted below)
    d["CQ"] = S // 512
    d["OWN"] = S // 2
    d["OTT"] = d["OWN"] // 128
    d["FT"] = F // 128
    d["VDT"] = H
    d["HG"] = d["HL"] // 2        # heads per exchange group (2 groups)
    assert S % 512 == 0 and D % 512 == 0 and F % 512 == 0
    return d


# ------------------------------------------------------------ host prep ---


def _bf(x):
    return np.asarray(x, np.float32).astype(ml_dtypes.bfloat16)


def _split3(v):
    """Split fp32 array (last axis vectors) into 3 bf16 rows summing to it."""
    v = np.asarray(v, np.float32)
    r0 = v.astype(ml_dtypes.bfloat16)
    rem = v - r0.astype(np.float32)
    r1 = rem.astype(ml_dtypes.bfloat16)
    r2 = (rem - r1.astype(np.float32)).astype(ml_dtypes.bfloat16)
    return np.stack([r0, r1, r2])


def make_core_inputs(cfg, inputs, core):
    c = _cfg_derived(cfg)
    S, D, F, H, HL, ST = c["S"], c["D"], c["F"], c["H"], c["HL"], c["ST"]
    HLW, OWN, FT = c["HLW"], c["OWN"], c["FT"]
    b, r = core // 2, core % 2
    hd = 128
    f32 = np.float32

    x = np.asarray(inputs["x"][b], f32)
    g1 = np.asarray(inputs["ln1_w"], f32)
    c1 = np.asarray(inputs["ln1_b"], f32)
    g2 = np.asarray(inputs["ln2_w"], f32)
    c2 = np.asarray(inputs["ln2_b"], f32)
    Wqkv = np.asarray(inputs["Wqkv"], f32)
    bqkv = np.asarray(inputs["bqkv"], f32)
    Wo = np.asarray(inputs["Wo"], f32)
    bo = np.asarray(inputs["bo"], f32)
    W1 = np.asarray(inputs["W1"], f32)
    b1 = np.asarray(inputs["b1"], f32)
    W2 = np.asarray(inputs["W2"], f32)
    b2 = np.asarray(inputs["b2"], f32)
    slopes = np.asarray(inputs["slopes"], f32)

    Wq, Wk, Wv = Wqkv[:D], Wqkv[D:2 * D], Wqkv[2 * D:]
    bq, bk, bv = bqkv[:D], bqkv[D:2 * D], bqkv[2 * D:]

    lo, hi = r * HLW, (r + 1) * HLW
    sc = 1.0 / np.sqrt(hd)

    Wq_l = Wq[lo:hi] * g1[None, :]
    Wk_l = Wk[lo:hi] * g1[None, :]
    Wv_l = Wv[lo:hi] * g1[None, :]
    qb = (Wq[lo:hi] @ c1 + bq[lo:hi]) * sc
    kb = Wk[lo:hi] @ c1 + bk[lo:hi]
    cbo = Wo @ (Wv @ c1 + bv) + bo          # v-bias + bo folded constant [D]

    W1p = W1 * g2[None, :]
    b1p = W1 @ c2 + b1

    heads = np.arange(r * HL, r * HL + HL)
    sl = slopes[heads]
    jpos = np.arange(S, dtype=f32)

    # exp bias table, half-step t grid: col (h, ti) = s_h*(j + 128*t - 64)
    # with t = -15 + 0.5*ti. The implied per-query-column scale
    # e^{s_h*(i-center)} is constant across key tiles for a fixed query
    # column, so it cancels between the AV accumulation and the denominator.
    # Steep-slope slots use 128-wide exp sub-chunks (range control); shallow
    # slots use 256/512-wide ones (fewer ACT instructions).
    NT = 37
    expbias = np.zeros((128, HL * NT), f32)
    for h in range(HL):
        for ti in range(NT):
            expbias[:, h * NT + ti] = sl[h] * (
                jpos[:128] + 128.0 * (-15.0 + 0.5 * ti) - 64.0)

    masks = np.zeros((128, 4 * 512), f32)
    jj = np.arange(128)[:, None]
    ii = np.arange(512)[None, :]
    for m in range(4):
        masks[:, m * 512:(m + 1) * 512] = np.where(m * 128 + jj <= ii, 0.0, NEG)

    return {
        "x_full": np.ascontiguousarray(_bf(x)),
        "x_own": np.ascontiguousarray(x[r * OWN:(r + 1) * OWN]),
        "wq_t": np.ascontiguousarray(_bf(Wq_l.T * sc)),
        "wk_t": np.ascontiguousarray(_bf(Wk_l.T)),
        "wv_t": np.ascontiguousarray(_bf(Wv_l.T)),
        "wo_t": np.ascontiguousarray(_bf(Wo.T)),
        "w1_t": np.ascontiguousarray(_bf(W1p.T)),
        "w2_t": np.ascontiguousarray(_bf(W2.T)),
        "qb": np.ascontiguousarray(qb.reshape(HL, 128).T),
        "kb": np.ascontiguousarray(kb.reshape(HL, 128).T),
        "b1c": np.ascontiguousarray(b1p.reshape(FT, 128).T),
        "obias": np.ascontiguousarray(_split3(cbo)[:2]),
        "b2row": np.ascontiguousarray(_split3(b2)[:2]),
        "expbias": expbias,
        "masks": np.ascontiguousarray(masks.astype(ml_dtypes.bfloat16)),
        "ident": np.eye(128, dtype=f32).astype(ml_dtypes.bfloat16),
        "sel": np.ascontiguousarray(
            np.repeat((np.arange(2) == r).astype(f32)[None, :], 128, axis=0)),
    }


# ------------------------------------------------------------ the kernel ---


def build_kernel(cfg, legalize=True):
    c = _cfg_derived(cfg)
    S, D, F = c["S"], c["D"], c["F"]
    HL, HLW, DT, ST = c["HL"], c["HLW"], c["DT"], c["ST"]
    CQ, QW, OWN, OTT, FT, VDT, HG = (c["CQ"], c["QW"], c["OWN"], c["OTT"],
                                     c["FT"], c["VDT"], c["HG"])
    GELU = getattr(AF, cfg["GELU"])

    # Single SWDGE sem lane: every DMA rides one FIFO queue (qPoolDynamic)
    # anyway, but Tile's default 8-lane round-robin sem assignment makes
    # slot-reuse DMAs wait on several DMASW sems at once, and the DMA ISA
    # encodes at most 2 waits (walrus "Too many sync wait commands").
    import concourse.tile_sem_assignment as tsa
    tsa.NUM_SWDGE_GLOBAL_SEMS = 1

    nc = bass.Bass()

    def param(name, shape, dt):
        return nc.declare_dram_parameter(name, shape, dt, isOutput=False)

    x_full_d = param("x_full", [S, D], BF16)
    x_own_d = param("x_own", [OWN, D], F32)
    wq_d = param("wq_t", [D, HLW], BF16)
    wk_d = param("wk_t", [D, HLW], BF16)
    wv_d = param("wv_t", [D, HLW], BF16)
    wo_d = param("wo_t", [D, D], BF16)
    w1_d = param("w1_t", [D, F], BF16)
    w2_d = param("w2_t", [F, D], BF16)
    qb_d = param("qb", [128, HL], F32)
    kb_d = param("kb", [128, HL], F32)
    b1c_d = param("b1c", [128, FT], F32)
    obias_d = param("obias", [2, D], BF16)
    b2row_d = param("b2row", [2, D], BF16)
    NT = 37
    expb_d = param("expbias", [128, HL * NT], F32)
    # exp sub-chunk width per head slot (slot h pairs global heads h and h+8;
    # the steeper slope 2^-(h+1)/2 of the pair bounds the exp argument range)
    GRAIN = [128, 256, 256, 512, 512, 512, 512, 512][:HL]
    masks_d = param("masks", [128, 4 * 512], BF16)
    ident_d = param("ident", [128, 128], BF16)
    sel_d = param("sel", [128, 2], F32)
    out_d = nc.declare_dram_parameter("out", [OWN, D], F32, isOutput=True)

    groups = [[0, 1], [2, 3], [4, 5], [6, 7]]
    # exchange chunk sizes (heads): small tail chunks so the last exchange
    # (the only exposed one) is cheap
    CHUNKS = [2, 2, 2, 1, 1] if HL == 8 else [1] * HL
    NG = len(CHUNKS)
    CH_OF = []                    # head -> chunk index
    for gi, n in enumerate(CHUNKS):
        CH_OF += [gi] * n
    CH_BASE = [sum(CHUNKS[:gi]) for gi in range(NG)]

    with tile.TileContext(nc) as tc, ExitStack() as top:
        def dma(out_ap, in_ap):
            nc.gpsimd.dma_start(out_ap, in_ap)

        def dma_sp(out_ap, in_ap):
            nc.sync.dma_start(out_ap, in_ap)

        def dma_blk_act(sbuf_ap, dram_ap):
            rows = dram_ap.shape[0]
            t = rows // 128
            nc.scalar.dma_start(
                sbuf_ap.rearrange("p (t f) -> p t f", t=t),
                dram_ap.rearrange("(t p) f -> p t f", p=128))

        def dma_blk(sbuf_ap, dram_ap):
            """DMA DRAM [T*128, N] into SBUF [128, T*N] (block t at t*N)."""
            rows = dram_ap.shape[0]
            t = rows // 128
            dma(sbuf_ap.rearrange("p (t f) -> p t f", t=t),
                dram_ap.rearrange("(t p) f -> p t f", p=128))

        dram = top.enter_context(tc.tile_pool(name="dram", bufs=1,
                                              space="DRAM"))
        exch_g = [dram.tile([2, CHUNKS[i] * 128, OWN], BF16,
                            name=f"exch{i}", tag=f"exch{i}")
                  for i in range(NG)]
        ago_g = [dram.tile([2, 2, CHUNKS[i] * 128, OWN], BF16,
                           name=f"ago{i}", tag=f"ago{i}") for i in range(NG)]
        x2buf = dram.tile([OWN, D], F32)

        es_a, es_qkv, es_wo, es_b = (ExitStack(), ExitStack(), ExitStack(),
                                     ExitStack())
        const = top.enter_context(tc.tile_pool(name="const", bufs=1))
        pool_a = es_a.enter_context(tc.tile_pool(name="slotA", bufs=1))

        # Packed constants: every tile pads to 4 KiB/partition, so the many
        # small tiles are packed into two wide ones and sliced by view.
        # bf16 pack: masks[0:2048] | ident[2048:2176] | ones[2176:2304]
        pk_b = const.tile([128, 4 * 512 + 128 + 128], BF16)
        masks = pk_b[:, 0:4 * 512]
        ident = pk_b[:, 4 * 512:4 * 512 + 128]
        _ones = pk_b[:, 4 * 512 + 128:4 * 512 + 256]
        ones3 = _ones[0:3, 0:128]
        ones2 = _ones[0:2, 0:128]
        onesc = _ones[:, 0:1]
        onesr = _ones[0:1, 0:128]
        # f32 pack: expb[0:HL*NT] | qb | kb | b1c | sel | epsc
        _c0 = HL * NT
        pk_f = const.tile([128, _c0 + 2 * HL + FT + 3], F32)
        expb = pk_f[:, 0:_c0]
        qb = pk_f[:, _c0:_c0 + HL]
        kb = pk_f[:, _c0 + HL:_c0 + 2 * HL]
        b1c = pk_f[:, _c0 + 2 * HL:_c0 + 2 * HL + FT]
        sel = pk_f[:, _c0 + 2 * HL + FT:_c0 + 2 * HL + FT + 2]
        epsc = pk_f[:, _c0 + 2 * HL + FT + 2:_c0 + 2 * HL + FT + 3]

        dma(ident, ident_d[:])
        nc.vector.memset(_ones, 1.0)
        nc.vector.memset(epsc, LN_EPS)
        dma(qb, qb_d[:])
        dma(kb, kb_d[:])

        # ---- LayerNorm helper (normalized output only; w/b pre-folded) ---
        LNG = D // 512 if D >= 512 else 1

        def layernorm_tile(stat, xt, out_bf):
            st = stat.tile([128, 6 * LNG + 4], F32, tag="lnstat")
            st6 = st[:, 0:6 * LNG]
            ag = st[:, 6 * LNG:6 * LNG + 2]
            sd = st[:, 6 * LNG + 2:6 * LNG + 3]
            rr = st[:, 6 * LNG + 3:6 * LNG + 4]
            for g in range(LNG):
                nc.vector.bn_stats(st6[:, 6 * g:6 * (g + 1)],
                                   xt[:, 512 * g:512 * (g + 1)])
            nc.vector.bn_aggr(ag, st6)
            nc.scalar.activation(sd, ag[:, 1:2], AF.Sqrt,
                                 bias=epsc, scale=1.0)
            nc.vector.reciprocal(rr, sd)
            nc.vector.tensor_scalar(
                out_bf[:], xt[:], scalar1=ag[:, 0:1], scalar2=rr,
                op0=ALU.subtract, op1=ALU.mult)

        # ---- phase A: LN1 + transpose into h_fm ---------------------------
        h_fm = pool_a.tile([128, DT * S], BF16, tag="a")
        with ExitStack() as ph:
            xpool = ph.enter_context(tc.tile_pool(name="ln_x", bufs=2))
            stat = ph.enter_context(tc.tile_pool(name="ln_stat", bufs=2))
            hbf = ph.enter_context(tc.tile_pool(name="ln_h", bufs=2))
            tps = ph.enter_context(
                tc.tile_pool(name="tps", bufs=6, space="PSUM"))
            for tt in range(ST):
                xt = xpool.tile([128, D], BF16)
                dma_sp(xt[:], x_full_d[tt * 128:(tt + 1) * 128, :])
                ht = hbf.tile([128, D], BF16)
                layernorm_tile(stat, xt, ht)
                for dt in range(DT):
                    tp = tps.tile([128, 128], BF16)
                    nc.tensor.transpose(
                        tp[:], ht[:, dt * 128:(dt + 1) * 128], ident)
                    dst = h_fm[:, dt * S + tt * 128: dt * S + (tt + 1) * 128]
                    if dt % 2 == 0:
                        nc.vector.tensor_copy(dst, tp[:])
                    else:
                        nc.scalar.copy(dst, tp[:])

        # ---- phase B: Q, K and V projections ------------------------------
        pool_q = es_qkv.enter_context(tc.tile_pool(name="slotQ", bufs=1, side="right"))
        pool_k = es_qkv.enter_context(tc.tile_pool(name="slotK", bufs=1, side="right"))
        pool_v = es_qkv.enter_context(tc.tile_pool(name="slotV", bufs=1, side="right"))
        q_sb = pool_q.tile([128, HL * S], BF16, tag="q")
        k_sb = pool_k.tile([128, HL * S], BF16, tag="k")
        v_sb = pool_v.tile([128, ST * HLW], BF16, tag="v")
        with ExitStack() as ph:
            mps = ph.enter_context(
                tc.tile_pool(name="bps", bufs=2, space="PSUM"))
            wpool = ph.enter_context(tc.tile_pool(name="slotW", bufs=1, side="right"))

            wk_sb = wpool.tile([128, DT * HLW], BF16, tag="w")
            dma_blk(wk_sb[:], wk_d.ap())
            for h in range(HL):
                for ch in range(CQ):
                    ps = mps.tile([128, QW], F32)
                    for dt in range(DT):
                        nc.tensor.matmul(
                            ps[:],
                            wk_sb[:, dt * HLW + h * 128:
                                  dt * HLW + (h + 1) * 128],
                            h_fm[:, dt * S + ch * QW: dt * S + (ch + 1) * QW],
                            start=(dt == 0), stop=(dt == DT - 1))
                    nc.vector.tensor_scalar_add(
                        k_sb[:, h * S + ch * QW: h * S + (ch + 1) * QW],
                        ps[:], kb[:, h:h + 1])

            wv_sb = wpool.tile([128, DT * HLW], BF16, tag="w")
            dma_blk(wv_sb[:], wv_d.ap())
            VCW = min(512, HLW)
            for jt in range(ST):
                for vc in range(HLW // VCW):
                    ps = mps.tile([128, VCW], F32)
                    for dt in range(DT):
                        nc.tensor.matmul(
                            ps[:],
                            h_fm[:, dt * S + jt * 128: dt * S + (jt + 1) * 128],
                            wv_sb[:, dt * HLW + vc * VCW:
                                  dt * HLW + (vc + 1) * VCW],
                            start=(dt == 0), stop=(dt == DT - 1))
                    nc.vector.tensor_copy(
                        v_sb[:, jt * HLW + vc * VCW:
                             jt * HLW + (vc + 1) * VCW],
                        ps[:])

            wq_sb = wpool.tile([128, DT * HLW], BF16, tag="w")
            dma_blk(wq_sb[:], wq_d.ap())
            for h in range(HL):
                for ch in range(CQ):
                    ps = mps.tile([128, QW], F32)
                    for dt in range(DT):
                        nc.tensor.matmul(
                            ps[:],
                            wq_sb[:, dt * HLW + h * 128:
                                  dt * HLW + (h + 1) * 128],
                            h_fm[:, dt * S + ch * QW: dt * S + (ch + 1) * QW],
                            start=(dt == 0), stop=(dt == DT - 1))
                    nc.vector.tensor_scalar_add(
                        q_sb[:, h * S + ch * QW: h * S + (ch + 1) * QW],
                        ps[:], qb[:, h:h + 1])

        es_a.close()   # h_fm released; wo_sb halves can land in its zone

        # ---- phase C: attention (head-outer) ------------------------------
        # Wo first half + obias prefetch: issued first so the 4 MiB load
        # streams while the attention loop runs (second half at phase E).
        pool_wo0 = es_wo.enter_context(tc.tile_pool(name="slotWo0", bufs=1))
        ob_p = es_wo.enter_context(tc.tile_pool(name="ob", bufs=1))
        HVD = (VDT // 2) * D
        wo_h0 = pool_wo0.tile([128, HVD], BF16, tag="wo0")
        dma_blk(wo_h0[:], wo_d[0:D // 2, :])
        obias = ob_p.tile([2, D], BF16)
        dma(obias[:], obias_d[:])
        dma(masks, masks_d[:])
        dma(expb, expb_d[:])
        dma(sel, sel_d[:])

        pool_c = es_wo.enter_context(tc.tile_pool(name="slotC", bufs=1))
        agp = es_wo.enter_context(tc.tile_pool(name="agp", bufs=2))
        attg = pool_c.tile([128, VDT * OWN], BF16, tag="c")

        def assemble(g):
            for sx in range(2):
                for hh2 in range(CHUNKS[g]):
                    h2 = CH_BASE[g] + hh2
                    g0 = agp.tile([128, OWN], BF16, tag="g")
                    dma_sp(g0[:], ago_g[g][sx, 0, hh2 * 128:(hh2 + 1) * 128, :])
                    g1 = agp.tile([128, OWN], BF16, tag="g")
                    dma_sp(g1[:], ago_g[g][sx, 1, hh2 * 128:(hh2 + 1) * 128, :])
                    dst = attg[:, (sx * HL + h2) * OWN:
                               (sx * HL + h2 + 1) * OWN]
                    nc.vector.tensor_scalar_mul(dst, g0[:], sel[:, 0:1])
                    nc.vector.scalar_tensor_tensor(
                        dst, g1[:], sel[:, 1:2], dst,
                        op0=ALU.mult, op1=ALU.add)

        with ExitStack() as ph:
            att_p = ph.enter_context(tc.tile_pool(name="att", bufs=3))
            bcn_p = ph.enter_context(tc.tile_pool(name="bcn", bufs=2))
            den_p = ph.enter_context(tc.tile_pool(name="den", bufs=2))
            oat_p = ph.enter_context(tc.tile_pool(name="oat", bufs=2))
            ps_s = ph.enter_context(
                tc.tile_pool(name="pss", bufs=3, space="PSUM"))
            ps_a = ph.enter_context(
                tc.tile_pool(name="psa", bufs=2, space="PSUM"))
            ps_d = ph.enter_context(
                tc.tile_pool(name="psd", bufs=2, space="PSUM"))
            ps_b = ph.enter_context(
                tc.tile_pool(name="psb", bufs=1, space="PSUM"))
            for h in range(HL):
                exch = exch_g[CH_OF[h]]
                hrow = (h - CH_BASE[CH_OF[h]]) * 128
                for ct in range(CQ):
                    njt = min(ST, (ct + 1) * (QW // 128))
                    pav = ps_a.tile([128, QW], F32)
                    pden = ps_d.tile([1, QW], F32)
                    for jt in range(njt):
                        pss = ps_s.tile([128, QW], F32)
                        nc.tensor.matmul(
                            pss[:],
                            k_sb[:, h * S + jt * 128: h * S + (jt + 1) * 128],
                            q_sb[:, h * S + ct * QW: h * S + (ct + 1) * QW],
                            start=True, stop=True)
                        m = jt - ct * (QW // 128)
                        if 0 <= m < 4:
                            nc.vector.tensor_add(
                                pss[:], pss[:],
                                masks[:, m * 512: m * 512 + QW])
                        at = att_p.tile([128, QW], BF16)
                        G = GRAIN[h]
                        e = jt - 4 * ct
                        for u in range(QW // G):
                            if G == 128:
                                ti = 2 * (e - u) + 30
                            elif G == 256:
                                ti = 2 * e - 4 * u - 1 + 30
                            else:
                                ti = 2 * e - 3 + 30
                            col = h * NT + ti
                            nc.scalar.activation(
                                at[:, u * G:(u + 1) * G],
                                pss[:, u * G:(u + 1) * G], AF.Exp,
                                bias=expb[:, col: col + 1],
                                scale=1.0)
                        nc.tensor.matmul(
                            pav[:],
                            v_sb[:, jt * HLW + h * 128:
                                 jt * HLW + (h + 1) * 128],
                            at[:], start=(jt == 0), stop=(jt == njt - 1))
                        nc.tensor.matmul(
                            pden[:], onesc, at[:],
                            start=(jt == 0), stop=(jt == njt - 1))
                    den = den_p.tile([1, 2 * QW], F32, tag="denf")
                    dsb = den[:, 0:QW]
                    rec = den[:, QW:2 * QW]
                    nc.vector.tensor_copy(dsb, pden[:])
                    nc.vector.reciprocal(rec, dsb)
                    recb = den_p.tile([1, QW], BF16, tag="denb")
                    nc.vector.tensor_copy(recb[:], rec)
                    pbc = ps_b.tile([128, QW], F32)
                    nc.tensor.matmul(pbc[:], onesr, recb[:],
                                     start=True, stop=True)
                    bcn = bcn_p.tile([128, QW], F32)
                    nc.vector.tensor_copy(bcn[:], pbc[:])
                    oat = oat_p.tile([128, QW], BF16)
                    nc.vector.scalar_tensor_tensor(
                        oat[:], pav[:], 1.0, bcn[:],
                        op0=ALU.mult, op1=ALU.mult)
                    for half in range(2):
                        a = max(ct * QW, half * OWN)
                        bnd = min((ct + 1) * QW, (half + 1) * OWN)
                        if a < bnd:
                            dma(
                                exch[half, hrow:hrow + 128,
                                     a - half * OWN: bnd - half * OWN],
                                oat[:, a - ct * QW: bnd - ct * QW])
                # chunk done -> exchange it under the remaining heads'
                # compute (only the last chunk's exchange is exposed)
                if h == HL - 1 or CH_OF[h + 1] != CH_OF[h]:
                    g = CH_OF[h]
                    nc.gpsimd.collective_compute(
                        "AllGather", ALU.bypass, replica_groups=groups,
                        ins=[exch_g[g].opt()], outs=[ago_g[g].opt()])
                    if g >= 1:
                        assemble(g - 1)
            assemble(NG - 1)

        es_qkv.close()

        # ---- phase E: out-proj + residual + LN2 + transpose --------------
        pool_wo1 = es_wo.enter_context(tc.tile_pool(name="slotWo1", bufs=1))
        wo_h1 = pool_wo1.tile([128, HVD], BF16, tag="wo1")
        dma_blk(wo_h1[:], wo_d[D // 2:D, :])

        w1_p = es_b.enter_context(tc.tile_pool(name="w1", bufs=2,
                                               side="right"))
        pool_b = es_b.enter_context(tc.tile_pool(name="slotB", bufs=1, side="right"))
        h2_fm = pool_b.tile([128, DT * OWN], BF16, tag="b")
        with ExitStack() as ph:
            xo_p = ph.enter_context(tc.tile_pool(name="xo", bufs=2))
            x2_p = ph.enter_context(tc.tile_pool(name="x2", bufs=2))
            h2_p = ph.enter_context(tc.tile_pool(name="h2", bufs=1))
            stat = ph.enter_context(tc.tile_pool(name="e_stat", bufs=2))
            ps_o = ph.enter_context(
                tc.tile_pool(name="pso", bufs=2, space="PSUM"))
            tps = ph.enter_context(
                tc.tile_pool(name="etps", bufs=6, space="PSUM"))

            for it in range(OTT):
                x2 = x2_p.tile([128, D], F32)
                for dc in range(D // 512):
                    po = ps_o.tile([128, 512], F32)
                    nc.tensor.matmul(
                        po[:], ones2, obias[:, dc * 512:(dc + 1) * 512],
                        start=True, stop=False)
                    for v in range(VDT):
                        wo_sb = wo_h0 if v < VDT // 2 else wo_h1
                        vv = v if v < VDT // 2 else v - VDT // 2
                        nc.tensor.matmul(
                            po[:],
                            attg[:, v * OWN + it * 128:
                                 v * OWN + (it + 1) * 128],
                            wo_sb[:, vv * D + dc * 512: vv * D + (dc + 1) * 512],
                            start=False, stop=(v == VDT - 1))
                    xo = xo_p.tile([128, 512], F32)
                    dma_sp(
                        xo[:],
                        x_own_d[it * 128:(it + 1) * 128,
                                dc * 512:(dc + 1) * 512])
                    nc.vector.tensor_add(
                        x2[:, dc * 512:(dc + 1) * 512], po[:], xo[:])
                dma_sp(x2buf[it * 128:(it + 1) * 128, :], x2[:])
                h2 = h2_p.tile([128, D], BF16)
                layernorm_tile(stat, x2, h2)
                for dt in range(DT):
                    tp = tps.tile([128, 128], BF16)
                    nc.tensor.transpose(
                        tp[:], h2[:, dt * 128:(dt + 1) * 128], ident)
                    dst2 = h2_fm[:, dt * OWN + it * 128:
                                 dt * OWN + (it + 1) * 128]
                    if dt % 2 == 0:
                        nc.vector.tensor_copy(dst2, tp[:])
                    else:
                        nc.scalar.copy(dst2, tp[:])

        es_wo.close()

        # ---- phase F: fused MLP — GELU output stays resident in SBUF -----
        # Per 512-token half: up-proj all F into g (f-partition layout, the
        # exact lhsT layout down-proj needs), then down-proj streaming w2 in
        # 256-col chunks. No DRAM round-trip for g; w1 loaded in 256-col
        # chunks (512B lines) instead of 128-col (256B lines).
        w2_p = top.enter_context(tc.tile_pool(name="w2", bufs=2))
        g_p = top.enter_context(tc.tile_pool(name="gsb", bufs=1))
        with ExitStack() as ph:
            b2_p = ph.enter_context(tc.tile_pool(name="b2", bufs=1))
            x2s_p = ph.enter_context(tc.tile_pool(name="x2s", bufs=2))
            o_p = ph.enter_context(tc.tile_pool(name="osb", bufs=2))
            ps_m = ph.enter_context(
                tc.tile_pool(name="psm", bufs=2, space="PSUM"))
            ps_d2 = ph.enter_context(
                tc.tile_pool(name="psd2", bufs=2, space="PSUM"))
            dma(b1c, b1c_d[:])
            b2row = b2_p.tile([2, D], BF16)
            dma(b2row[:], b2row_d[:])
            HTOK = 512                      # tokens per fused half
            W1C = 512                       # w1 f-cols per load
            W2C = 256                       # w2 d-cols per load
            for hf in range(OWN // HTOK):
                g = g_p.tile([128, FT * HTOK], BF16, tag="g")
                for fc in range(F // W1C):
                    w1t = w1_p.tile([128, DT * W1C], BF16)
                    dma_blk(w1t[:], w1_d[:, fc * W1C:(fc + 1) * W1C])
                    for sub in range(W1C // 128):
                        ft = fc * (W1C // 128) + sub
                        ps = ps_m.tile([128, HTOK], F32)
                        for dt in range(DT):
                            nc.tensor.matmul(
                                ps[:],
                                w1t[:, dt * W1C + sub * 128:
                                    dt * W1C + (sub + 1) * 128],
                                h2_fm[:, dt * OWN + hf * HTOK:
                                      dt * OWN + (hf + 1) * HTOK],
                                start=(dt == 0), stop=(dt == DT - 1))
                        nc.scalar.activation(
                            g[:, ft * HTOK:(ft + 1) * HTOK], ps[:], GELU,
                            bias=b1c[:, ft:ft + 1], scale=1.0)
                for dc in range(D // W2C):
                    w2t = w2_p.tile([128, FT * W2C], BF16)
                    dma_blk(w2t[:], w2_d[:, dc * W2C:(dc + 1) * W2C])
                    for it2 in range(HTOK // 128):
                        it = hf * (HTOK // 128) + it2
                        ps = ps_d2.tile([128, W2C], F32)
                        nc.tensor.matmul(
                            ps[:], ones2, b2row[:, dc * W2C:(dc + 1) * W2C],
                            start=True, stop=False)
                        for ft in range(FT):
                            nc.tensor.matmul(
                                ps[:],
                                g[:, ft * HTOK + it2 * 128:
                                    ft * HTOK + (it2 + 1) * 128],
                                w2t[:, ft * W2C:(ft + 1) * W2C],
                                start=False, stop=(ft == FT - 1))
                        x2t = x2s_p.tile([128, W2C], F32)
                        dma_sp(x2t[:],
                            x2buf[it * 128:(it + 1) * 128,
                                  dc * W2C:(dc + 1) * W2C])
                        ot = o_p.tile([128, W2C], F32)
                        nc.vector.tensor_add(ot[:], ps[:], x2t[:])
                        dma_sp(out_d[it * 128:(it + 1) * 128,
                                     dc * W2C:(dc + 1) * W2C],
                               ot[:])

        es_b.close()

    if legalize:
        _legalize_waits(nc)
    return nc


def _legalize_waits(nc):
    """walrus on this container encodes at most ONE sync wait per DMA/branch
    instruction. Tile emits several (reader-WAR + DMA-lane WAW). Waits are
    executed by the issuing engine's sequencer in program order, so hoisting
    all-but-one wait onto wait-only EventSemaphore instructions inserted
    immediately before it on the same engine stream is semantics-preserving."""
    n_split = 0
    for fn in nc.m.functions:
        for bb in fn.blocks:
            out = []
            for inst in bb.instructions:
                si = inst.sync_info
                waits = list(si.on_wait) if si and si.on_wait else []
                if len(waits) > 1:
                    # merge same-sem waits to the max value
                    merged = {}
                    for w in waits:
                        k = (w.sync_type, w.id, w.wait_mode)
                        if k not in merged or merged[k].wait_value < w.wait_value:
                            merged[k] = w
                    waits = list(merged.values())
                    for w in waits[:-1]:
                        es = mybir.InstEventSemaphore(
                            name=f"{inst.name}-wsplit{n_split}",
                            engine=inst.engine,
                            ins=[], outs=[],
                            sync_info=mybir.SyncInfo(on_wait=[w], on_update=[]),
                        )
                        out.append(es)
                        n_split += 1
                    inst.sync_info = mybir.SyncInfo(
                        on_wait=[waits[-1]],
                        on_update=list(si.on_update) if si.on_update else [])
                out.append(inst)
            bb.instructions[:] = out


# ------------------------------------------------------------- the entry ---

_BUILT = {}


def _get_nc(cfg_key=None):
    if "nc" not in _BUILT:
        _BUILT["nc"] = build_kernel(REAL_CFG)
    return _BUILT["nc"]


def kernel(**inputs):
    cfg = REAL_CFG
    c = _cfg_derived(cfg)
    nc = _get_nc()
    in_maps = [make_core_inputs(cfg, inputs, core) for core in range(8)]
    from concourse.bass_utils import run_bass_kernel_spmd
    res = run_bass_kernel_spmd(nc, in_maps, list(range(8)))
    B = np.asarray(inputs["x"]).shape[0]
    S, D, OWN = cfg["S"], cfg["D"], c["OWN"]
    out = np.empty((B, S, D), np.float32)
    for core in range(8):
        b, r = core // 2, core % 2
        out[b, r * OWN:(r + 1) * OWN, :] = res.results[core]["out"]
    return out



# revision 3
# speedup vs baseline: 1.3961x; 1.0003x over previous
"""Trainium2 Bass kernel for a pre-LN transformer block (dense_transformer).

Reference computation (fp32, per batch element):
    x = x + Attn(LN1(x));  x = x + MLP(LN2(x))
with 16-head causal ALiBi attention (S=2048, D=2048) and a 4*D GELU MLP.

Distribution: 4 batches x 2-way head/tensor parallel = 8 cores.
Core c handles batch c//2 with pair-rank r=c%2:
  - attention: 8 local heads (r*8..r*8+7), all 2048 query positions. Scores
    are computed transposed [j(key) x i(query)]; the full ALiBi+softmax term
    rides the ACT-exp per-partition bias alone: the bias encodes
    s_h*(j - center_u) for a per-query sub-chunk center, and the implied
    per-query-column scale e^{s_h*(i-center_u)} is constant across key tiles
    so it cancels between the AV accumulation and the softmax denominator.
    No seed matmul is needed. Steep-slope head slots use 128-wide exp
    sub-chunks (fp32/bf16 range control); shallow slots use 256/512-wide
    ones (fewer ACT instructions). Head slot grain is compile-time; which
    head lives in a slot is per-core DATA, so the stream stays SPMD.
  - softmax denominators via ones-lhsT matmuls accumulated alongside AV;
    normalization is fused into the AV PSUM->SBUF copy using a K=1 broadcast
    matmul of the reciprocal row.
  - a pair AllGather (chunked by head group) swaps attention halves so each
    core owns 1024 tokens for the output projection, residual, LN2 and MLP;
    gathered chunks are assembled into the Wo operand DURING the remaining
    heads' attention compute (only the last 1-head chunk is exposed).
All per-core variation (weight slices, ALiBi slopes, token offsets) is input
DATA; the instruction stream is identical on all 8 cores (SPMD).

v3 structural changes vs v2 (same math):
  - fused MLP: GELU output g stays resident in SBUF in f-partition layout
    (exactly the lhsT layout the down-projection needs), processed in two
    512-token halves; the 48 MiB DRAM round-trip of v2 (gbuf write + narrow
    256B-line reload) is gone. w1 streams in 512-col chunks (1KB DMA lines,
    ~3x the measured single-queue bandwidth of 256B lines), w2 in 256-col
    chunks double-buffered.
  - two DMA queues: residual/x traffic, attention-output exchange writes and
    the Wo halves ride the SP engine's hardware-DGE queue; weight streams,
    collectives and AllGather-result reads keep the gpsimd software-DGE
    queue (assembly reads sit right behind their collective there, so they
    never head-of-line-block anything). This removes FIFO blocking between
    independent streams (all DMA previously serialized on one queue).
    (Do NOT issue DMAs from the ACT queue: their sync waits stall the ACT
    sequencer and delay GELU/exp work behind them - measured regression.)
  - head loop order interleaves the exp-heavy slots (PI permutation) so the
    ACT engine's exp load is spread across the attention phase instead of
    front-loaded; exchange buffers are position-indexed, attg slot-indexed.
  - x streamed in bf16 for the LN1 pass (residual path keeps fp32 x_own),
    halving the startup DMA and doubling DVE throughput there; transpose
    PSUM->SBUF copies alternate DVE/ACT.
  - scores PSUM pool depth 3 so the PE can run ahead of ACT exp.

Measured on 8xTRN2 (slope method, amortizing the ~85 ms axon round-trip and
~0.5 ms per-exec runtime overhead): v2 3.77 ms -> v3 2.43 ms per execution
(T(17)-T(1) lever arm; shorter arms read 0.1-0.2 ms lower on lucky RTTs).

The walrus in this container encodes at most ONE sync wait per instruction,
so _legalize_waits() splits every multi-wait instruction into wait-only
EventSemaphore instructions inserted immediately before it on the same
engine stream - order-preserving, so semantics are unchanged.
"""

import os
import sys

for _p in ("/opt/trn_rl_repo", "/opt/trn_rl_repo/concourse"):
    if os.path.isdir(_p) and _p not in sys.path:
        sys.path.append(_p)

import numpy as np
import ml_dtypes

import concourse.bass as bass
import concourse.mybir as mybir
import concourse.tile as tile
from contextlib import ExitStack

BF16 = mybir.dt.bfloat16
F32 = mybir.dt.float32
AF = mybir.ActivationFunctionType
ALU = mybir.AluOpType

REAL_CFG = dict(S=2048, D=2048, F=8192, H=16, GELU="Gelu")
LN_EPS = 1e-5
NEG = -1.0e6  # causal mask additive value (pre-exp)


def _cfg_derived(cfg):
    S, D, F, H = cfg["S"], cfg["D"], cfg["F"], cfg["H"]
    d = dict(cfg)
    d["HL"] = H // 2              # local heads per core
    d["HLW"] = d["HL"] * 128      # local head width (vd)
    d["DT"] = D // 128
    d["ST"] = S // 128
    d["QW"] = 512                 # q-chunk width (asserted below)
    d["CQ"] = S // 512
    d["OWN"] = S // 2
    d["OTT"] = d["OWN"] // 128
    d["FT"] = F // 128
    d["VDT"] = H
    d["HG"] = d["HL"] // 2        # heads per exchange group (2 groups)
    assert S % 512 == 0 and D % 512 == 0 and F % 512 == 0
    return d


# ------------------------------------------------------------ host prep ---


def _bf(x):
    return np.asarray(x, np.float32).astype(ml_dtypes.bfloat16)


def _split3(v):
    """Split fp32 array (last axis vectors) into 3 bf16 rows summing to it."""
    v = np.asarray(v, np.float32)
    r0 = v.astype(ml_dtypes.bfloat16)
    rem = v - r0.astype(np.float32)
    r1 = rem.astype(ml_dtypes.bfloat16)
    r2 = (rem - r1.astype(np.float32)).astype(ml_dtypes.bfloat16)
    return np.stack([r0, r1, r2])


def make_core_inputs(cfg, inputs, core):
    c = _cfg_derived(cfg)
    S, D, F, H, HL, ST = c["S"], c["D"], c["F"], c["H"], c["HL"], c["ST"]
    HLW, OWN, FT = c["HLW"], c["OWN"], c["FT"]
    b, r = core // 2, core % 2
    hd = 128
    f32 = np.float32

    x = np.asarray(inputs["x"][b], f32)
    g1 = np.asarray(inputs["ln1_w"], f32)
    c1 = np.asarray(inputs["ln1_b"], f32)
    g2 = np.asarray(inputs["ln2_w"], f32)
    c2 = np.asarray(inputs["ln2_b"], f32)
    Wqkv = np.asarray(inputs["Wqkv"], f32)
    bqkv = np.asarray(inputs["bqkv"], f32)
    Wo = np.asarray(inputs["Wo"], f32)
    bo = np.asarray(inputs["bo"], f32)
    W1 = np.asarray(inputs["W1"], f32)
    b1 = np.asarray(inputs["b1"], f32)
    W2 = np.asarray(inputs["W2"], f32)
    b2 = np.asarray(inputs["b2"], f32)
    slopes = np.asarray(inputs["slopes"], f32)

    Wq, Wk, Wv = Wqkv[:D], Wqkv[D:2 * D], Wqkv[2 * D:]
    bq, bk, bv = bqkv[:D], bqkv[D:2 * D], bqkv[2 * D:]

    lo, hi = r * HLW, (r + 1) * HLW
    sc = 1.0 / np.sqrt(hd)

    Wq_l = Wq[lo:hi] * g1[None, :]
    Wk_l = Wk[lo:hi] * g1[None, :]
    Wv_l = Wv[lo:hi] * g1[None, :]
    qb = (Wq[lo:hi] @ c1 + bq[lo:hi]) * sc
    kb = Wk[lo:hi] @ c1 + bk[lo:hi]
    cbo = Wo @ (Wv @ c1 + bv) + bo          # v-bias + bo folded constant [D]

    W1p = W1 * g2[None, :]
    b1p = W1 @ c2 + b1

    heads = np.arange(r * HL, r * HL + HL)
    sl = slopes[heads]
    jpos = np.arange(S, dtype=f32)

    # exp bias table, half-step t grid: col (h, ti) = s_h*(j + 128*t - 64)
    # with t = -15 + 0.5*ti. The implied per-query-column scale
    # e^{s_h*(i-center)} is constant across key tiles for a fixed query
    # column, so it cancels between the AV accumulation and the denominator.
    # Steep-slope slots use 128-wide exp sub-chunks (range control); shallow
    # slots use 256/512-wide ones (fewer ACT instructions).
    NT = 37
    expbias = np.zeros((128, HL * NT), f32)
    for h in range(HL):
        for ti in range(NT):
            expbias[:, h * NT + ti] = sl[h] * (
                jpos[:128] + 128.0 * (-15.0 + 0.5 * ti) - 64.0)

    masks = np.zeros((128, 4 * 512), f32)
    jj = np.arange(128)[:, None]
    ii = np.arange(512)[None, :]
    for m in range(4):
        masks[:, m * 512:(m + 1) * 512] = np.where(m * 128 + jj <= ii, 0.0, NEG)

    return {
        "x_full": np.ascontiguousarray(_bf(x)),
        "x_own": np.ascontiguousarray(x[r * OWN:(r + 1) * OWN]),
        "wq_t": np.ascontiguousarray(_bf(Wq_l.T * sc)),
        "wk_t": np.ascontiguousarray(_bf(Wk_l.T)),
        "wv_t": np.ascontiguousarray(_bf(Wv_l.T)),
        "wo_t": np.ascontiguousarray(_bf(Wo.T)),
        "w1_t": np.ascontiguousarray(_bf(W1p.T)),
        "w2_t": np.ascontiguousarray(_bf(W2.T)),
        "qb": np.ascontiguousarray(qb.reshape(HL, 128).T),
        "kb": np.ascontiguousarray(kb.reshape(HL, 128).T),
        "b1c": np.ascontiguousarray(b1p.reshape(FT, 128).T),
        "obias": np.ascontiguousarray(_split3(cbo)[:2]),
        "b2row": np.ascontiguousarray(_split3(b2)[:2]),
        "expbias": expbias,
        "masks": np.ascontiguousarray(masks.astype(ml_dtypes.bfloat16)),
        "ident": np.eye(128, dtype=f32).astype(ml_dtypes.bfloat16),
        "sel": np.ascontiguousarray(
            np.repeat((np.arange(2) == r).astype(f32)[None, :], 128, axis=0)),
    }


# ------------------------------------------------------------ the kernel ---


def build_kernel(cfg, legalize=True):
    c = _cfg_derived(cfg)
    S, D, F = c["S"], c["D"], c["F"]
    HL, HLW, DT, ST = c["HL"], c["HLW"], c["DT"], c["ST"]
    CQ, QW, OWN, OTT, FT, VDT, HG = (c["CQ"], c["QW"], c["OWN"], c["OTT"],
                                     c["FT"], c["VDT"], c["HG"])
    GELU = getattr(AF, cfg["GELU"])

    # Single SWDGE sem lane: every DMA rides one FIFO queue (qPoolDynamic)
    # anyway, but Tile's default 8-lane round-robin sem assignment makes
    # slot-reuse DMAs wait on several DMASW sems at once, and the DMA ISA
    # encodes at most 2 waits (walrus "Too many sync wait commands").
    import concourse.tile_sem_assignment as tsa
    tsa.NUM_SWDGE_GLOBAL_SEMS = 1

    nc = bass.Bass()

    def param(name, shape, dt):
        return nc.declare_dram_parameter(name, shape, dt, isOutput=False)

    x_full_d = param("x_full", [S, D], BF16)
    x_own_d = param("x_own", [OWN, D], F32)
    wq_d = param("wq_t", [D, HLW], BF16)
    wk_d = param("wk_t", [D, HLW], BF16)
    wv_d = param("wv_t", [D, HLW], BF16)
    wo_d = param("wo_t", [D, D], BF16)
    w1_d = param("w1_t", [D, F], BF16)
    w2_d = param("w2_t", [F, D], BF16)
    qb_d = param("qb", [128, HL], F32)
    kb_d = param("kb", [128, HL], F32)
    b1c_d = param("b1c", [128, FT], F32)
    obias_d = param("obias", [2, D], BF16)
    b2row_d = param("b2row", [2, D], BF16)
    NT = 37
    expb_d = param("expbias", [128, HL * NT], F32)
    # exp sub-chunk width per head slot (slot h pairs global heads h and h+8;
    # the steeper slope 2^-(h+1)/2 of the pair bounds the exp argument range)
    GRAIN = [128, 256, 256, 512, 512, 512, 512, 512][:HL]
    masks_d = param("masks", [128, 4 * 512], BF16)
    ident_d = param("ident", [128, 128], BF16)
    sel_d = param("sel", [128, 2], F32)
    out_d = nc.declare_dram_parameter("out", [OWN, D], F32, isOutput=True)

    groups = [[0, 1], [2, 3], [4, 5], [6, 7]]
    # exchange chunk sizes (heads): small tail chunks so the last exchange
    # (the only exposed one) is cheap
    CHUNKS = [2, 2, 2, 1, 1] if HL == 8 else [1] * HL
    NG = len(CHUNKS)
    CH_OF = []                    # head -> chunk index
    for gi, n in enumerate(CHUNKS):
        CH_OF += [gi] * n
    CH_BASE = [sum(CHUNKS[:gi]) for gi in range(NG)]

    with tile.TileContext(nc) as tc, ExitStack() as top:
        def dma(out_ap, in_ap):
            nc.gpsimd.dma_start(out_ap, in_ap)

        def dma_sp(out_ap, in_ap):
            nc.sync.dma_start(out_ap, in_ap)

        def dma_blk_sp(sbuf_ap, dram_ap):
            rows = dram_ap.shape[0]
            t = rows // 128
            nc.sync.dma_start(
                sbuf_ap.rearrange("p (t f) -> p t f", t=t),
                dram_ap.rearrange("(t p) f -> p t f", p=128))

        def dma_blk_act(sbuf_ap, dram_ap):
            rows = dram_ap.shape[0]
            t = rows // 128
            nc.scalar.dma_start(
                sbuf_ap.rearrange("p (t f) -> p t f", t=t),
                dram_ap.rearrange("(t p) f -> p t f", p=128))

        def dma_blk(sbuf_ap, dram_ap):
            """DMA DRAM [T*128, N] into SBUF [128, T*N] (block t at t*N)."""
            rows = dram_ap.shape[0]
            t = rows // 128
            dma(sbuf_ap.rearrange("p (t f) -> p t f", t=t),
                dram_ap.rearrange("(t p) f -> p t f", p=128))

        dram = top.enter_context(tc.tile_pool(name="dram", bufs=1,
                                              space="DRAM"))
        exch_g = [dram.tile([2, CHUNKS[i] * 128, OWN], BF16,
                            name=f"exch{i}", tag=f"exch{i}")
                  for i in range(NG)]
        ago_g = [dram.tile([2, 2, CHUNKS[i] * 128, OWN], BF16,
                           name=f"ago{i}", tag=f"ago{i}") for i in range(NG)]
        x2buf = dram.tile([OWN, D], F32)

        es_a, es_qkv, es_wo, es_b = (ExitStack(), ExitStack(), ExitStack(),
                                     ExitStack())
        const = top.enter_context(tc.tile_pool(name="const", bufs=1))
        pool_a = es_a.enter_context(tc.tile_pool(name="slotA", bufs=1))

        # Packed constants: every tile pads to 4 KiB/partition, so the many
        # small tiles are packed into two wide ones and sliced by view.
        # bf16 pack: masks[0:2048] | ident[2048:2176] | ones[2176:2304]
        pk_b = const.tile([128, 4 * 512 + 128 + 128], BF16)
        masks = pk_b[:, 0:4 * 512]
        ident = pk_b[:, 4 * 512:4 * 512 + 128]
        _ones = pk_b[:, 4 * 512 + 128:4 * 512 + 256]
        ones3 = _ones[0:3, 0:128]
        ones2 = _ones[0:2, 0:128]
        onesc = _ones[:, 0:1]
        onesr = _ones[0:1, 0:128]
        # f32 pack: expb[0:HL*NT] | qb | kb | b1c | sel | epsc
        _c0 = HL * NT
        pk_f = const.tile([128, _c0 + 2 * HL + FT + 3], F32)
        expb = pk_f[:, 0:_c0]
        qb = pk_f[:, _c0:_c0 + HL]
        kb = pk_f[:, _c0 + HL:_c0 + 2 * HL]
        b1c = pk_f[:, _c0 + 2 * HL:_c0 + 2 * HL + FT]
        sel = pk_f[:, _c0 + 2 * HL + FT:_c0 + 2 * HL + FT + 2]
        epsc = pk_f[:, _c0 + 2 * HL + FT + 2:_c0 + 2 * HL + FT + 3]

        dma(ident, ident_d[:])
        nc.vector.memset(_ones, 1.0)
        nc.vector.memset(epsc, LN_EPS)
        dma(qb, qb_d[:])
        dma(kb, kb_d[:])

        # ---- LayerNorm helper (normalized output only; w/b pre-folded) ---
        LNG = D // 512 if D >= 512 else 1

        def layernorm_tile(stat, xt, out_bf):
            st = stat.tile([128, 6 * LNG + 4], F32, tag="lnstat")
            st6 = st[:, 0:6 * LNG]
            ag = st[:, 6 * LNG:6 * LNG + 2]
            sd = st[:, 6 * LNG + 2:6 * LNG + 3]
            rr = st[:, 6 * LNG + 3:6 * LNG + 4]
            for g in range(LNG):
                nc.vector.bn_stats(st6[:, 6 * g:6 * (g + 1)],
                                   xt[:, 512 * g:512 * (g + 1)])
            nc.vector.bn_aggr(ag, st6)
            nc.scalar.activation(sd, ag[:, 1:2], AF.Sqrt,
                                 bias=epsc, scale=1.0)
            nc.vector.reciprocal(rr, sd)
            nc.vector.tensor_scalar(
                out_bf[:], xt[:], scalar1=ag[:, 0:1], scalar2=rr,
                op0=ALU.subtract, op1=ALU.mult)

        # ---- phase A: LN1 + transpose into h_fm ---------------------------
        h_fm = pool_a.tile([128, DT * S], BF16, tag="a")
        with ExitStack() as ph:
            xpool = ph.enter_context(tc.tile_pool(name="ln_x", bufs=3))
            stat = ph.enter_context(tc.tile_pool(name="ln_stat", bufs=3))
            hbf = ph.enter_context(tc.tile_pool(name="ln_h", bufs=3))
            tps = ph.enter_context(
                tc.tile_pool(name="tps", bufs=6, space="PSUM"))
            for tt in range(ST):
                xt = xpool.tile([128, D], BF16)
                dma_sp(xt[:], x_full_d[tt * 128:(tt + 1) * 128, :])
                ht = hbf.tile([128, D], BF16)
                layernorm_tile(stat, xt, ht)
                for dt in range(DT):
                    tp = tps.tile([128, 128], BF16)
                    nc.tensor.transpose(
                        tp[:], ht[:, dt * 128:(dt + 1) * 128], ident)
                    dst = h_fm[:, dt * S + tt * 128: dt * S + (tt + 1) * 128]
                    if dt % 2 == 0:
                        nc.vector.tensor_copy(dst, tp[:])
                    else:
                        nc.scalar.copy(dst, tp[:])

        # ---- phase B: Q, K and V projections ------------------------------
        pool_q = es_qkv.enter_context(tc.tile_pool(name="slotQ", bufs=1, side="right"))
        pool_k = es_qkv.enter_context(tc.tile_pool(name="slotK", bufs=1, side="right"))
        pool_v = es_qkv.enter_context(tc.tile_pool(name="slotV", bufs=1, side="right"))
        q_sb = pool_q.tile([128, HL * S], BF16, tag="q")
        k_sb = pool_k.tile([128, HL * S], BF16, tag="k")
        v_sb = pool_v.tile([128, ST * HLW], BF16, tag="v")
        with ExitStack() as ph:
            mps = ph.enter_context(
                tc.tile_pool(name="bps", bufs=2, space="PSUM"))
            wpool = ph.enter_context(tc.tile_pool(name="slotW", bufs=1, side="right"))

            wk_sb = wpool.tile([128, DT * HLW], BF16, tag="w")
            dma_blk(wk_sb[:], wk_d.ap())
            for h in range(HL):
                for ch in range(CQ):
                    ps = mps.tile([128, QW], F32)
                    for dt in range(DT):
                        nc.tensor.matmul(
                            ps[:],
                            wk_sb[:, dt * HLW + h * 128:
                                  dt * HLW + (h + 1) * 128],
                            h_fm[:, dt * S + ch * QW: dt * S + (ch + 1) * QW],
                            start=(dt == 0), stop=(dt == DT - 1))
                    nc.vector.tensor_scalar_add(
                        k_sb[:, h * S + ch * QW: h * S + (ch + 1) * QW],
                        ps[:], kb[:, h:h + 1])

            wv_sb = wpool.tile([128, DT * HLW], BF16, tag="w")
            dma_blk(wv_sb[:], wv_d.ap())
            VCW = min(512, HLW)
            for jt in range(ST):
                for vc in range(HLW // VCW):
                    ps = mps.tile([128, VCW], F32)
                    for dt in range(DT):
                        nc.tensor.matmul(
                            ps[:],
                            h_fm[:, dt * S + jt * 128: dt * S + (jt + 1) * 128],
                            wv_sb[:, dt * HLW + vc * VCW:
                                  dt * HLW + (vc + 1) * VCW],
                            start=(dt == 0), stop=(dt == DT - 1))
                    nc.vector.tensor_copy(
                        v_sb[:, jt * HLW + vc * VCW:
                             jt * HLW + (vc + 1) * VCW],
                        ps[:])

            wq_sb = wpool.tile([128, DT * HLW], BF16, tag="w")
            dma_blk(wq_sb[:], wq_d.ap())
            for h in range(HL):
                for ch in range(CQ):
                    ps = mps.tile([128, QW], F32)
                    for dt in range(DT):
                        nc.tensor.matmul(
                            ps[:],
                            wq_sb[:, dt * HLW + h * 128:
                                  dt * HLW + (h + 1) * 128],
                            h_fm[:, dt * S + ch * QW: dt * S + (ch + 1) * QW],
                            start=(dt == 0), stop=(dt == DT - 1))
                    nc.vector.tensor_scalar_add(
                        q_sb[:, h * S + ch * QW: h * S + (ch + 1) * QW],
                        ps[:], qb[:, h:h + 1])

        es_a.close()   # h_fm released; wo_sb halves can land in its zone

        # ---- phase C: attention (head-outer) ------------------------------
        # Wo first half + obias prefetch: issued first so the 4 MiB load
        # streams while the attention loop runs (second half at phase E).
        pool_wo0 = es_wo.enter_context(tc.tile_pool(name="slotWo0", bufs=1))
        ob_p = es_wo.enter_context(tc.tile_pool(name="ob", bufs=1))
        HVD = (VDT // 2) * D
        wo_h0 = pool_wo0.tile([128, HVD], BF16, tag="wo0")
        dma_blk_sp(wo_h0[:], wo_d[0:D // 2, :])
        obias = ob_p.tile([2, D], BF16)
        dma(obias[:], obias_d[:])
        dma(masks, masks_d[:])
        dma(expb, expb_d[:])
        dma(sel, sel_d[:])

        pool_c = es_wo.enter_context(tc.tile_pool(name="slotC", bufs=1))
        agp = es_wo.enter_context(tc.tile_pool(name="agp", bufs=2))
        attg = pool_c.tile([128, VDT * OWN], BF16, tag="c")

        # loop-order permutation: spread the 4-way-exp slots (0,1) and the
        # 2-way ones (2,3) across the head loop so the ACT engine's exp load
        # stays even instead of front-loaded. Pure bookkeeping: exchange
        # buffers are position-indexed, attg stays slot-indexed.
        PI = [0, 4, 2, 5, 1, 6, 3, 7][:HL] if HL == 8 else list(range(HL))

        def assemble(g):
            for sx in range(2):
                for hh2 in range(CHUNKS[g]):
                    h2 = PI[CH_BASE[g] + hh2]
                    g0 = agp.tile([128, OWN], BF16, tag="g")
                    dma(g0[:], ago_g[g][sx, 0, hh2 * 128:(hh2 + 1) * 128, :])
                    g1 = agp.tile([128, OWN], BF16, tag="g")
                    dma(g1[:], ago_g[g][sx, 1, hh2 * 128:(hh2 + 1) * 128, :])
                    dst = attg[:, (sx * HL + h2) * OWN:
                               (sx * HL + h2 + 1) * OWN]
                    nc.vector.tensor_scalar_mul(dst, g0[:], sel[:, 0:1])
                    nc.vector.scalar_tensor_tensor(
                        dst, g1[:], sel[:, 1:2], dst,
                        op0=ALU.mult, op1=ALU.add)

        with ExitStack() as ph:
            att_p = ph.enter_context(tc.tile_pool(name="att", bufs=4))
            bcn_p = ph.enter_context(tc.tile_pool(name="bcn", bufs=2))
            den_p = ph.enter_context(tc.tile_pool(name="den", bufs=2))
            oat_p = ph.enter_context(tc.tile_pool(name="oat", bufs=2))
            ps_s = ph.enter_context(
                tc.tile_pool(name="pss", bufs=3, space="PSUM"))
            ps_a = ph.enter_context(
                tc.tile_pool(name="psa", bufs=2, space="PSUM"))
            ps_d = ph.enter_context(
                tc.tile_pool(name="psd", bufs=2, space="PSUM"))
            ps_b = ph.enter_context(
                tc.tile_pool(name="psb", bufs=1, space="PSUM"))
            for hi in range(HL):
                h = PI[hi]
                exch = exch_g[CH_OF[hi]]
                hrow = (hi - CH_BASE[CH_OF[hi]]) * 128
                for ct in range(CQ):
                    njt = min(ST, (ct + 1) * (QW // 128))
                    pav = ps_a.tile([128, QW], F32)
                    pden = ps_d.tile([1, QW], F32)
                    for jt in range(njt):
                        pss = ps_s.tile([128, QW], F32)
                        nc.tensor.matmul(
                            pss[:],
                            k_sb[:, h * S + jt * 128: h * S + (jt + 1) * 128],
                            q_sb[:, h * S + ct * QW: h * S + (ct + 1) * QW],
                            start=True, stop=True)
                        m = jt - ct * (QW // 128)
                        if 0 <= m < 4:
                            nc.vector.tensor_add(
                                pss[:], pss[:],
                                masks[:, m * 512: m * 512 + QW])
                        at = att_p.tile([128, QW], BF16)
                        G = GRAIN[h]
                        e = jt - 4 * ct
                        for u in range(QW // G):
                            if G == 128:
                                ti = 2 * (e - u) + 30
                            elif G == 256:
                                ti = 2 * e - 4 * u - 1 + 30
                            else:
                                ti = 2 * e - 3 + 30
                            col = h * NT + ti
                            nc.scalar.activation(
                                at[:, u * G:(u + 1) * G],
                                pss[:, u * G:(u + 1) * G], AF.Exp,
                                bias=expb[:, col: col + 1],
                                scale=1.0)
                        nc.tensor.matmul(
                            pav[:],
                            v_sb[:, jt * HLW + h * 128:
                                 jt * HLW + (h + 1) * 128],
                            at[:], start=(jt == 0), stop=(jt == njt - 1))
                        nc.tensor.matmul(
                            pden[:], onesc, at[:],
                            start=(jt == 0), stop=(jt == njt - 1))
                    den = den_p.tile([1, 2 * QW], F32, tag="denf")
                    dsb = den[:, 0:QW]
                    rec = den[:, QW:2 * QW]
                    nc.vector.tensor_copy(dsb, pden[:])
                    nc.vector.reciprocal(rec, dsb)
                    recb = den_p.tile([1, QW], BF16, tag="denb")
                    nc.vector.tensor_copy(recb[:], rec)
                    pbc = ps_b.tile([128, QW], F32)
                    nc.tensor.matmul(pbc[:], onesr, recb[:],
                                     start=True, stop=True)
                    bcn = bcn_p.tile([128, QW], F32)
                    nc.vector.tensor_copy(bcn[:], pbc[:])
                    oat = oat_p.tile([128, QW], BF16)
                    nc.vector.scalar_tensor_tensor(
                        oat[:], pav[:], 1.0, bcn[:],
                        op0=ALU.mult, op1=ALU.mult)
                    for half in range(2):
                        a = max(ct * QW, half * OWN)
                        bnd = min((ct + 1) * QW, (half + 1) * OWN)
                        if a < bnd:
                            dma_sp(
                                exch[half, hrow:hrow + 128,
                                     a - half * OWN: bnd - half * OWN],
                                oat[:, a - ct * QW: bnd - ct * QW])
                # chunk done -> exchange it under the remaining heads'
                # compute (only the last chunk's exchange is exposed)
                if hi == HL - 1 or CH_OF[hi + 1] != CH_OF[hi]:
                    g = CH_OF[hi]
                    nc.gpsimd.collective_compute(
                        "AllGather", ALU.bypass, replica_groups=groups,
                        ins=[exch_g[g].opt()], outs=[ago_g[g].opt()])
                    if g >= 1:
                        assemble(g - 1)
            assemble(NG - 1)

        es_qkv.close()

        # ---- phase E: out-proj + residual + LN2 + transpose --------------
        pool_wo1 = es_wo.enter_context(tc.tile_pool(name="slotWo1", bufs=1))
        wo_h1 = pool_wo1.tile([128, HVD], BF16, tag="wo1")
        dma_blk_sp(wo_h1[:], wo_d[D // 2:D, :])

        w1_p = es_b.enter_context(tc.tile_pool(name="w1", bufs=2,
                                               side="right"))
        pool_b = es_b.enter_context(tc.tile_pool(name="slotB", bufs=1, side="right"))
        h2_fm = pool_b.tile([128, DT * OWN], BF16, tag="b")
        with ExitStack() as ph:
            xo_p = ph.enter_context(tc.tile_pool(name="xo", bufs=2))
            x2_p = ph.enter_context(tc.tile_pool(name="x2", bufs=2))
            h2_p = ph.enter_context(tc.tile_pool(name="h2", bufs=1))
            stat = ph.enter_context(tc.tile_pool(name="e_stat", bufs=2))
            ps_o = ph.enter_context(
                tc.tile_pool(name="pso", bufs=2, space="PSUM"))
            tps = ph.enter_context(
                tc.tile_pool(name="etps", bufs=6, space="PSUM"))

            for it in range(OTT):
                x2 = x2_p.tile([128, D], F32)
                for dc in range(D // 512):
                    po = ps_o.tile([128, 512], F32)
                    nc.tensor.matmul(
                        po[:], ones2, obias[:, dc * 512:(dc + 1) * 512],
                        start=True, stop=False)
                    for v in range(VDT):
                        wo_sb = wo_h0 if v < VDT // 2 else wo_h1
                        vv = v if v < VDT // 2 else v - VDT // 2
                        nc.tensor.matmul(
                            po[:],
                            attg[:, v * OWN + it * 128:
                                 v * OWN + (it + 1) * 128],
                            wo_sb[:, vv * D + dc * 512: vv * D + (dc + 1) * 512],
                            start=False, stop=(v == VDT - 1))
                    xo = xo_p.tile([128, 512], F32)
                    dma_sp(
                        xo[:],
                        x_own_d[it * 128:(it + 1) * 128,
                                dc * 512:(dc + 1) * 512])
                    nc.vector.tensor_add(
                        x2[:, dc * 512:(dc + 1) * 512], po[:], xo[:])
                dma_sp(x2buf[it * 128:(it + 1) * 128, :], x2[:])
                h2 = h2_p.tile([128, D], BF16)
                layernorm_tile(stat, x2, h2)
                for dt in range(DT):
                    tp = tps.tile([128, 128], BF16)
                    nc.tensor.transpose(
                        tp[:], h2[:, dt * 128:(dt + 1) * 128], ident)
                    dst2 = h2_fm[:, dt * OWN + it * 128:
                                 dt * OWN + (it + 1) * 128]
                    if dt % 2 == 0:
                        nc.vector.tensor_copy(dst2, tp[:])
                    else:
                        nc.scalar.copy(dst2, tp[:])

        es_wo.close()

        # ---- phase F: fused MLP — GELU output stays resident in SBUF -----
        # Per 512-token half: up-proj all F into g (f-partition layout, the
        # exact lhsT layout down-proj needs), then down-proj streaming w2 in
        # 256-col chunks. No DRAM round-trip for g; w1 loaded in 256-col
        # chunks (512B lines) instead of 128-col (256B lines).
        w2_p = top.enter_context(tc.tile_pool(name="w2", bufs=2))
        g_p = top.enter_context(tc.tile_pool(name="gsb", bufs=1))
        with ExitStack() as ph:
            b2_p = ph.enter_context(tc.tile_pool(name="b2", bufs=1))
            x2s_p = ph.enter_context(tc.tile_pool(name="x2s", bufs=2))
            o_p = ph.enter_context(tc.tile_pool(name="osb", bufs=2))
            ps_m = ph.enter_context(
                tc.tile_pool(name="psm", bufs=2, space="PSUM"))
            ps_d2 = ph.enter_context(
                tc.tile_pool(name="psd2", bufs=2, space="PSUM"))
            dma(b1c, b1c_d[:])
            b2row = b2_p.tile([2, D], BF16)
            dma(b2row[:], b2row_d[:])
            HTOK = 512                      # tokens per fused half
            W1C = 512                       # w1 f-cols per load
            W2C = 256                       # w2 d-cols per load
            for hf in range(OWN // HTOK):
                g = g_p.tile([128, FT * HTOK], BF16, tag="g")
                for fc in range(F // W1C):
                    w1t = w1_p.tile([128, DT * W1C], BF16)
                    dma_blk(w1t[:], w1_d[:, fc * W1C:(fc + 1) * W1C])
                    for sub in range(W1C // 128):
                        ft = fc * (W1C // 128) + sub
                        ps = ps_m.tile([128, HTOK], F32)
                        for dt in range(DT):
                            nc.tensor.matmul(
                                ps[:],
                                w1t[:, dt * W1C + sub * 128:
                                    dt * W1C + (sub + 1) * 128],
                                h2_fm[:, dt * OWN + hf * HTOK:
                                      dt * OWN + (hf + 1) * HTOK],
                                start=(dt == 0), stop=(dt == DT - 1))
                        nc.scalar.activation(
                            g[:, ft * HTOK:(ft + 1) * HTOK], ps[:], GELU,
                            bias=b1c[:, ft:ft + 1], scale=1.0)
                for dc in range(D // W2C):
                    w2t = w2_p.tile([128, FT * W2C], BF16)
                    dma_blk(w2t[:], w2_d[:, dc * W2C:(dc + 1) * W2C])
                    for it2 in range(HTOK // 128):
                        it = hf * (HTOK // 128) + it2
                        ps = ps_d2.tile([128, W2C], F32)
                        nc.tensor.matmul(
                            ps[:], ones2, b2row[:, dc * W2C:(dc + 1) * W2C],
                            start=True, stop=False)
                        for ft in range(FT):
                            nc.tensor.matmul(
                                ps[:],
                                g[:, ft * HTOK + it2 * 128:
                                    ft * HTOK + (it2 + 1) * 128],
                                w2t[:, ft * W2C:(ft + 1) * W2C],
                                start=False, stop=(ft == FT - 1))
                        x2t = x2s_p.tile([128, W2C], F32)
                        dma_sp(x2t[:],
                            x2buf[it * 128:(it + 1) * 128,
                                  dc * W2C:(dc + 1) * W2C])
                        ot = o_p.tile([128, W2C], F32)
                        nc.vector.tensor_add(ot[:], ps[:], x2t[:])
                        dma_sp(out_d[it * 128:(it + 1) * 128,
                                     dc * W2C:(dc + 1) * W2C],
                               ot[:])

        es_b.close()

    if legalize:
        _legalize_waits(nc)
    return nc


def _legalize_waits(nc):
    """walrus on this container encodes at most ONE sync wait per DMA/branch
    instruction. Tile emits several (reader-WAR + DMA-lane WAW). Waits are
    executed by the issuing engine's sequencer in program order, so hoisting
    all-but-one wait onto wait-only EventSemaphore instructions inserted
    immediately before it on the same engine stream is semantics-preserving."""
    n_split = 0
    for fn in nc.m.functions:
        for bb in fn.blocks:
            out = []
            for inst in bb.instructions:
                si = inst.sync_info
                waits = list(si.on_wait) if si and si.on_wait else []
                if len(waits) > 1:
                    # merge same-sem waits to the max value
                    merged = {}
                    for w in waits:
                        k = (w.sync_type, w.id, w.wait_mode)
                        if k not in merged or merged[k].wait_value < w.wait_value:
                            merged[k] = w
                    waits = list(merged.values())
                    for w in waits[:-1]:
                        es = mybir.InstEventSemaphore(
                            name=f"{inst.name}-wsplit{n_split}",
                            engine=inst.engine,
                            ins=[], outs=[],
                            sync_info=mybir.SyncInfo(on_wait=[w], on_update=[]),
                        )
                        out.append(es)
                        n_split += 1
                    inst.sync_info = mybir.SyncInfo(
                        on_wait=[waits[-1]],
                        on_update=list(si.on_update) if si.on_update else [])
                out.append(inst)
            bb.instructions[:] = out


# ------------------------------------------------------------- the entry ---

_BUILT = {}


def _get_nc(cfg_key=None):
    if "nc" not in _BUILT:
        _BUILT["nc"] = build_kernel(REAL_CFG)
    return _BUILT["nc"]


def kernel(**inputs):
    cfg = REAL_CFG
    c = _cfg_derived(cfg)
    nc = _get_nc()
    in_maps = [make_core_inputs(cfg, inputs, core) for core in range(8)]
    from concourse.bass_utils import run_bass_kernel_spmd
    res = run_bass_kernel_spmd(nc, in_maps, list(range(8)))
    B = np.asarray(inputs["x"]).shape[0]
    S, D, OWN = cfg["S"], cfg["D"], c["OWN"]
    out = np.empty((B, S, D), np.float32)
    for core in range(8):
        b, r = core // 2, core % 2
        out[b, r * OWN:(r + 1) * OWN, :] = res.results[core]["out"]
    return out



# revision 5
# speedup vs baseline: 1.4110x; 1.0107x over previous
"""Trainium2 Bass kernel for a pre-LN transformer block (dense_transformer).

Reference computation (fp32, per batch element):
    x = x + Attn(LN1(x));  x = x + MLP(LN2(x))
with 16-head causal ALiBi attention (S=2048, D=2048) and a 4*D GELU MLP.

Distribution: 4 batches x 2-way head/tensor parallel = 8 cores.
Core c handles batch c//2 with pair-rank r=c%2:
  - attention: 8 local heads (r*8..r*8+7), all 2048 query positions. Scores
    are computed transposed [j(key) x i(query)]; the full ALiBi+softmax term
    rides the ACT-exp per-partition bias alone: the bias encodes
    s_h*(j - center_u) for a per-query sub-chunk center, and the implied
    per-query-column scale e^{s_h*(i-center_u)} is constant across key tiles
    so it cancels between the AV accumulation and the softmax denominator.
    No seed matmul is needed. Steep-slope head slots use 128-wide exp
    sub-chunks (fp32/bf16 range control); shallow slots use 256/512-wide
    ones (fewer ACT instructions). Head slot grain is compile-time; which
    head lives in a slot is per-core DATA, so the stream stays SPMD.
  - softmax denominators via ones-lhsT matmuls accumulated alongside AV;
    normalization is fused into the AV PSUM->SBUF copy using a K=1 broadcast
    matmul of the reciprocal row.
  - a pair AllGather (chunked by head group) swaps attention halves so each
    core owns 1024 tokens for the output projection, residual, LN2 and MLP;
    gathered chunks are assembled into the Wo operand DURING the remaining
    heads' attention compute (only the last 1-head chunk is exposed).
All per-core variation (weight slices, ALiBi slopes, token offsets) is input
DATA; the instruction stream is identical on all 8 cores (SPMD).

v3 structural changes vs v2 (same math):
  - fused MLP: GELU output g stays resident in SBUF in f-partition layout
    (exactly the lhsT layout the down-projection needs), processed in two
    512-token halves; the 48 MiB DRAM round-trip of v2 (gbuf write + narrow
    256B-line reload) is gone. w1 streams in 512-col chunks (1KB DMA lines,
    ~3x the measured single-queue bandwidth of 256B lines), w2 in 256-col
    chunks double-buffered.
  - two DMA queues: residual/x traffic, attention-output exchange writes and
    the Wo halves ride the SP engine's hardware-DGE queue; weight streams,
    collectives and AllGather-result reads keep the gpsimd software-DGE
    queue (assembly reads sit right behind their collective there, so they
    never head-of-line-block anything). This removes FIFO blocking between
    independent streams (all DMA previously serialized on one queue).
    (Do NOT issue DMAs from the ACT queue: their sync waits stall the ACT
    sequencer and delay GELU/exp work behind them - measured regression.)
  - head loop order interleaves the exp-heavy slots (PI permutation) so the
    ACT engine's exp load is spread across the attention phase instead of
    front-loaded; exchange buffers are position-indexed, attg slot-indexed.
  - x streamed in bf16 for the LN1 pass (residual path keeps fp32 x_own),
    halving the startup DMA and doubling DVE throughput there; transpose
    PSUM->SBUF copies alternate DVE/ACT.
  - scores PSUM pool depth 3 so the PE can run ahead of ACT exp.

Measured on 8xTRN2 (slope method, amortizing the ~85 ms axon round-trip and
~0.5 ms per-exec runtime overhead): v2 3.77 ms -> v3 2.43 ms per execution
(T(17)-T(1) lever arm; shorter arms read 0.1-0.2 ms lower on lucky RTTs).

The walrus in this container encodes at most ONE sync wait per instruction,
so _legalize_waits() splits every multi-wait instruction into wait-only
EventSemaphore instructions inserted immediately before it on the same
engine stream - order-preserving, so semantics are unchanged.
"""

import os
import sys

for _p in ("/opt/trn_rl_repo", "/opt/trn_rl_repo/concourse"):
    if os.path.isdir(_p) and _p not in sys.path:
        sys.path.append(_p)

import numpy as np
import ml_dtypes

import concourse.bass as bass
import concourse.mybir as mybir
import concourse.tile as tile
from contextlib import ExitStack

BF16 = mybir.dt.bfloat16
F32 = mybir.dt.float32
AF = mybir.ActivationFunctionType
ALU = mybir.AluOpType

REAL_CFG = dict(S=2048, D=2048, F=8192, H=16, GELU="Gelu")
LN_EPS = 1e-5
NEG = -1.0e6  # causal mask additive value (pre-exp)


def _cfg_derived(cfg):
    S, D, F, H = cfg["S"], cfg["D"], cfg["F"], cfg["H"]
    d = dict(cfg)
    d["HL"] = H // 2              # local heads per core
    d["HLW"] = d["HL"] * 128      # local head width (vd)
    d["DT"] = D // 128
    d["ST"] = S // 128
    d["QW"] = 512                 # q-chunk width (asserted below)
    d["CQ"] = S // 512
    d["OWN"] = S // 2
    d["OTT"] = d["OWN"] // 128
    d["FT"] = F // 128
    d["VDT"] = H
    d["HG"] = d["HL"] // 2        # heads per exchange group (2 groups)
    assert S % 512 == 0 and D % 512 == 0 and F % 512 == 0
    return d


# ------------------------------------------------------------ host prep ---


def _bf(x):
    return np.asarray(x, np.float32).astype(ml_dtypes.bfloat16)


def _split3(v):
    """Split fp32 array (last axis vectors) into 3 bf16 rows summing to it."""
    v = np.asarray(v, np.float32)
    r0 = v.astype(ml_dtypes.bfloat16)
    rem = v - r0.astype(np.float32)
    r1 = rem.astype(ml_dtypes.bfloat16)
    r2 = (rem - r1.astype(np.float32)).astype(ml_dtypes.bfloat16)
    return np.stack([r0, r1, r2])


def make_core_inputs(cfg, inputs, core):
    c = _cfg_derived(cfg)
    S, D, F, H, HL, ST = c["S"], c["D"], c["F"], c["H"], c["HL"], c["ST"]
    HLW, OWN, FT = c["HLW"], c["OWN"], c["FT"]
    b, r = core // 2, core % 2
    hd = 128
    f32 = np.float32

    x = np.asarray(inputs["x"][b], f32)
    g1 = np.asarray(inputs["ln1_w"], f32)
    c1 = np.asarray(inputs["ln1_b"], f32)
    g2 = np.asarray(inputs["ln2_w"], f32)
    c2 = np.asarray(inputs["ln2_b"], f32)
    Wqkv = np.asarray(inputs["Wqkv"], f32)
    bqkv = np.asarray(inputs["bqkv"], f32)
    Wo = np.asarray(inputs["Wo"], f32)
    bo = np.asarray(inputs["bo"], f32)
    W1 = np.asarray(inputs["W1"], f32)
    b1 = np.asarray(inputs["b1"], f32)
    W2 = np.asarray(inputs["W2"], f32)
    b2 = np.asarray(inputs["b2"], f32)
    slopes = np.asarray(inputs["slopes"], f32)

    Wq, Wk, Wv = Wqkv[:D], Wqkv[D:2 * D], Wqkv[2 * D:]
    bq, bk, bv = bqkv[:D], bqkv[D:2 * D], bqkv[2 * D:]

    lo, hi = r * HLW, (r + 1) * HLW
    sc = 1.0 / np.sqrt(hd)

    Wq_l = Wq[lo:hi] * g1[None, :]
    Wk_l = Wk[lo:hi] * g1[None, :]
    Wv_l = Wv[lo:hi] * g1[None, :]
    qb = (Wq[lo:hi] @ c1 + bq[lo:hi]) * sc
    kb = Wk[lo:hi] @ c1 + bk[lo:hi]
    cbo = Wo @ (Wv @ c1 + bv) + bo          # v-bias + bo folded constant [D]

    W1p = W1 * g2[None, :]
    b1p = W1 @ c2 + b1

    heads = np.arange(r * HL, r * HL + HL)
    sl = slopes[heads]
    jpos = np.arange(S, dtype=f32)

    # exp bias table, half-step t grid: col (h, ti) = s_h*(j + 128*t - 64)
    # with t = -15 + 0.5*ti. The implied per-query-column scale
    # e^{s_h*(i-center)} is constant across key tiles for a fixed query
    # column, so it cancels between the AV accumulation and the denominator.
    # Steep-slope slots use 128-wide exp sub-chunks (range control); shallow
    # slots use 256/512-wide ones (fewer ACT instructions).
    NT = 37
    expbias = np.zeros((128, HL * NT), f32)
    for h in range(HL):
        for ti in range(NT):
            expbias[:, h * NT + ti] = sl[h] * (
                jpos[:128] + 128.0 * (-15.0 + 0.5 * ti) - 64.0)

    masks = np.zeros((128, 4 * 512), f32)
    jj = np.arange(128)[:, None]
    ii = np.arange(512)[None, :]
    for m in range(4):
        masks[:, m * 512:(m + 1) * 512] = np.where(m * 128 + jj <= ii, 0.0, NEG)

    return {
        "x_full": np.ascontiguousarray(_bf(x)),
        "x_own": np.ascontiguousarray(x[r * OWN:(r + 1) * OWN]),
        "wq_t": np.ascontiguousarray(_bf(Wq_l.T * sc)),
        "wk_t": np.ascontiguousarray(_bf(Wk_l.T)),
        "wv_t": np.ascontiguousarray(_bf(Wv_l.T)),
        "wo_t": np.ascontiguousarray(_bf(Wo.T)),
        "w1_t": np.ascontiguousarray(_bf(W1p.T)),
        "w2_t": np.ascontiguousarray(_bf(W2.T)),
        "qb": np.ascontiguousarray(qb.reshape(HL, 128).T),
        "kb": np.ascontiguousarray(kb.reshape(HL, 128).T),
        "b1c": np.ascontiguousarray(b1p.reshape(FT, 128).T),
        "obias": np.ascontiguousarray(_split3(cbo)[:2]),
        "b2row": np.ascontiguousarray(_split3(b2)[:2]),
        "expbias": expbias,
        "masks": np.ascontiguousarray(masks.astype(ml_dtypes.bfloat16)),
        "ident": np.eye(128, dtype=f32).astype(ml_dtypes.bfloat16),
        "sel": np.ascontiguousarray(
            np.repeat((np.arange(2) == r).astype(f32)[None, :], 128, axis=0)),
    }


# ------------------------------------------------------------ the kernel ---


def build_kernel(cfg, legalize=True):
    c = _cfg_derived(cfg)
    S, D, F = c["S"], c["D"], c["F"]
    HL, HLW, DT, ST = c["HL"], c["HLW"], c["DT"], c["ST"]
    CQ, QW, OWN, OTT, FT, VDT, HG = (c["CQ"], c["QW"], c["OWN"], c["OTT"],
                                     c["FT"], c["VDT"], c["HG"])
    GELU = getattr(AF, cfg["GELU"])

    # Single SWDGE sem lane: every DMA rides one FIFO queue (qPoolDynamic)
    # anyway, but Tile's default 8-lane round-robin sem assignment makes
    # slot-reuse DMAs wait on several DMASW sems at once, and the DMA ISA
    # encodes at most 2 waits (walrus "Too many sync wait commands").
    import concourse.tile_sem_assignment as tsa
    tsa.NUM_SWDGE_GLOBAL_SEMS = 1

    nc = bass.Bass()

    def param(name, shape, dt):
        return nc.declare_dram_parameter(name, shape, dt, isOutput=False)

    x_full_d = param("x_full", [S, D], BF16)
    x_own_d = param("x_own", [OWN, D], F32)
    wq_d = param("wq_t", [D, HLW], BF16)
    wk_d = param("wk_t", [D, HLW], BF16)
    wv_d = param("wv_t", [D, HLW], BF16)
    wo_d = param("wo_t", [D, D], BF16)
    w1_d = param("w1_t", [D, F], BF16)
    w2_d = param("w2_t", [F, D], BF16)
    qb_d = param("qb", [128, HL], F32)
    kb_d = param("kb", [128, HL], F32)
    b1c_d = param("b1c", [128, FT], F32)
    obias_d = param("obias", [2, D], BF16)
    b2row_d = param("b2row", [2, D], BF16)
    NT = 37
    expb_d = param("expbias", [128, HL * NT], F32)
    # exp sub-chunk width per head slot (slot h pairs global heads h and h+8;
    # the steeper slope 2^-(h+1)/2 of the pair bounds the exp argument range)
    GRAIN = [128, 256, 256, 512, 512, 512, 512, 512][:HL]
    masks_d = param("masks", [128, 4 * 512], BF16)
    ident_d = param("ident", [128, 128], BF16)
    sel_d = param("sel", [128, 2], F32)
    out_d = nc.declare_dram_parameter("out", [OWN, D], F32, isOutput=True)

    groups = [[0, 1], [2, 3], [4, 5], [6, 7]]
    # exchange chunk sizes (heads): small tail chunks so the last exchange
    # (the only exposed one) is cheap
    CHUNKS = [2, 2, 2, 1, 1] if HL == 8 else [1] * HL
    NG = len(CHUNKS)
    CH_OF = []                    # head -> chunk index
    for gi, n in enumerate(CHUNKS):
        CH_OF += [gi] * n
    CH_BASE = [sum(CHUNKS[:gi]) for gi in range(NG)]

    with tile.TileContext(nc) as tc, ExitStack() as top:
        def dma(out_ap, in_ap):
            nc.gpsimd.dma_start(out_ap, in_ap)

        def dma_sp(out_ap, in_ap):
            nc.sync.dma_start(out_ap, in_ap)

        def dma_blk_sp(sbuf_ap, dram_ap):
            rows = dram_ap.shape[0]
            t = rows // 128
            nc.sync.dma_start(
                sbuf_ap.rearrange("p (t f) -> p t f", t=t),
                dram_ap.rearrange("(t p) f -> p t f", p=128))

        def dma_blk_act(sbuf_ap, dram_ap):
            rows = dram_ap.shape[0]
            t = rows // 128
            nc.scalar.dma_start(
                sbuf_ap.rearrange("p (t f) -> p t f", t=t),
                dram_ap.rearrange("(t p) f -> p t f", p=128))

        def dma_blk(sbuf_ap, dram_ap):
            """DMA DRAM [T*128, N] into SBUF [128, T*N] (block t at t*N)."""
            rows = dram_ap.shape[0]
            t = rows // 128
            dma(sbuf_ap.rearrange("p (t f) -> p t f", t=t),
                dram_ap.rearrange("(t p) f -> p t f", p=128))

        dram = top.enter_context(tc.tile_pool(name="dram", bufs=1,
                                              space="DRAM"))
        exch_g = [dram.tile([2, CHUNKS[i] * 128, OWN], BF16,
                            name=f"exch{i}", tag=f"exch{i}")
                  for i in range(NG)]
        ago_g = [dram.tile([2, 2, CHUNKS[i] * 128, OWN], BF16,
                           name=f"ago{i}", tag=f"ago{i}") for i in range(NG)]
        x2buf = dram.tile([OWN, D], F32)

        es_a, es_qkv, es_wo, es_b = (ExitStack(), ExitStack(), ExitStack(),
                                     ExitStack())
        const = top.enter_context(tc.tile_pool(name="const", bufs=1))
        pool_a = es_a.enter_context(tc.tile_pool(name="slotA", bufs=1))

        # Packed constants: every tile pads to 4 KiB/partition, so the many
        # small tiles are packed into two wide ones and sliced by view.
        # bf16 pack: masks[0:2048] | ident[2048:2176] | ones[2176:2304]
        pk_b = const.tile([128, 4 * 512 + 128 + 128], BF16)
        masks = pk_b[:, 0:4 * 512]
        ident = pk_b[:, 4 * 512:4 * 512 + 128]
        _ones = pk_b[:, 4 * 512 + 128:4 * 512 + 256]
        ones3 = _ones[0:3, 0:128]
        ones2 = _ones[0:2, 0:128]
        onesc = _ones[:, 0:1]
        onesr = _ones[0:1, 0:128]
        # f32 pack: expb[0:HL*NT] | qb | kb | b1c | sel | epsc
        _c0 = HL * NT
        pk_f = const.tile([128, _c0 + 2 * HL + FT + 3], F32)
        expb = pk_f[:, 0:_c0]
        qb = pk_f[:, _c0:_c0 + HL]
        kb = pk_f[:, _c0 + HL:_c0 + 2 * HL]
        b1c = pk_f[:, _c0 + 2 * HL:_c0 + 2 * HL + FT]
        sel = pk_f[:, _c0 + 2 * HL + FT:_c0 + 2 * HL + FT + 2]
        epsc = pk_f[:, _c0 + 2 * HL + FT + 2:_c0 + 2 * HL + FT + 3]

        dma(ident, ident_d[:])
        nc.vector.memset(_ones, 1.0)
        nc.vector.memset(epsc, LN_EPS)
        dma(qb, qb_d[:])
        dma(kb, kb_d[:])

        # ---- LayerNorm helper (normalized output only; w/b pre-folded) ---
        LNG = D // 512 if D >= 512 else 1

        def layernorm_tile(stat, xt, out_bf):
            st = stat.tile([128, 6 * LNG + 4], F32, tag="lnstat")
            st6 = st[:, 0:6 * LNG]
            ag = st[:, 6 * LNG:6 * LNG + 2]
            sd = st[:, 6 * LNG + 2:6 * LNG + 3]
            rr = st[:, 6 * LNG + 3:6 * LNG + 4]
            for g in range(LNG):
                nc.vector.bn_stats(st6[:, 6 * g:6 * (g + 1)],
                                   xt[:, 512 * g:512 * (g + 1)])
            nc.vector.bn_aggr(ag, st6)
            nc.scalar.activation(sd, ag[:, 1:2], AF.Sqrt,
                                 bias=epsc, scale=1.0)
            nc.vector.reciprocal(rr, sd)
            nc.vector.tensor_scalar(
                out_bf[:], xt[:], scalar1=ag[:, 0:1], scalar2=rr,
                op0=ALU.subtract, op1=ALU.mult)

        # ---- phase A: LN1 + transpose into h_fm ---------------------------
        h_fm = pool_a.tile([128, DT * S], BF16, tag="a")
        with ExitStack() as ph:
            xpool = ph.enter_context(tc.tile_pool(name="ln_x", bufs=3))
            stat = ph.enter_context(tc.tile_pool(name="ln_stat", bufs=3))
            hbf = ph.enter_context(tc.tile_pool(name="ln_h", bufs=3))
            tps = ph.enter_context(
                tc.tile_pool(name="tps", bufs=6, space="PSUM"))
            for tt in range(ST):
                xt = xpool.tile([128, D], BF16)
                dma_sp(xt[:], x_full_d[tt * 128:(tt + 1) * 128, :])
                ht = hbf.tile([128, D], BF16)
                layernorm_tile(stat, xt, ht)
                for dt in range(DT):
                    tp = tps.tile([128, 128], BF16)
                    nc.tensor.transpose(
                        tp[:], ht[:, dt * 128:(dt + 1) * 128], ident)
                    dst = h_fm[:, dt * S + tt * 128: dt * S + (tt + 1) * 128]
                    if dt % 2 == 0:
                        nc.vector.tensor_copy(dst, tp[:])
                    else:
                        nc.scalar.copy(dst, tp[:])

        # ---- phase B: Q, K and V projections ------------------------------
        pool_q = es_qkv.enter_context(tc.tile_pool(name="slotQ", bufs=1, side="right"))
        pool_k = es_qkv.enter_context(tc.tile_pool(name="slotK", bufs=1, side="right"))
        pool_v = es_qkv.enter_context(tc.tile_pool(name="slotV", bufs=1, side="right"))
        q_sb = pool_q.tile([128, HL * S], BF16, tag="q")
        k_sb = pool_k.tile([128, HL * S], BF16, tag="k")
        v_sb = pool_v.tile([128, ST * HLW], BF16, tag="v")
        with ExitStack() as ph:
            mps = ph.enter_context(
                tc.tile_pool(name="bps", bufs=4, space="PSUM"))
            wpool = ph.enter_context(tc.tile_pool(name="slotW", bufs=1, side="right"))

            wv_sb = wpool.tile([128, DT * HLW], BF16, tag="w")
            dma_blk(wv_sb[:], wv_d.ap())
            VCW = min(512, HLW)
            for jt in range(ST):
                for vc in range(HLW // VCW):
                    ps = mps.tile([128, VCW], F32)
                    for dt in range(DT):
                        nc.tensor.matmul(
                            ps[:],
                            h_fm[:, dt * S + jt * 128: dt * S + (jt + 1) * 128],
                            wv_sb[:, dt * HLW + vc * VCW:
                                  dt * HLW + (vc + 1) * VCW],
                            start=(dt == 0), stop=(dt == DT - 1))
                    nc.vector.tensor_copy(
                        v_sb[:, jt * HLW + vc * VCW:
                             jt * HLW + (vc + 1) * VCW],
                        ps[:])

            wk_sb = wpool.tile([128, DT * HLW], BF16, tag="w")
            dma_blk(wk_sb[:], wk_d.ap())
            for h in range(HL):
                for ch in range(CQ):
                    ps = mps.tile([128, QW], F32)
                    for dt in range(DT):
                        nc.tensor.matmul(
                            ps[:],
                            wk_sb[:, dt * HLW + h * 128:
                                  dt * HLW + (h + 1) * 128],
                            h_fm[:, dt * S + ch * QW: dt * S + (ch + 1) * QW],
                            start=(dt == 0), stop=(dt == DT - 1))
                    nc.vector.tensor_scalar_add(
                        k_sb[:, h * S + ch * QW: h * S + (ch + 1) * QW],
                        ps[:], kb[:, h:h + 1])

            wq_sb = wpool.tile([128, DT * HLW], BF16, tag="w")
            dma_blk(wq_sb[:], wq_d.ap())
            for h in range(HL):
                for ch in range(CQ):
                    ps = mps.tile([128, QW], F32)
                    for dt in range(DT):
                        nc.tensor.matmul(
                            ps[:],
                            wq_sb[:, dt * HLW + h * 128:
                                  dt * HLW + (h + 1) * 128],
                            h_fm[:, dt * S + ch * QW: dt * S + (ch + 1) * QW],
                            start=(dt == 0), stop=(dt == DT - 1))
                    nc.vector.tensor_scalar_add(
                        q_sb[:, h * S + ch * QW: h * S + (ch + 1) * QW],
                        ps[:], qb[:, h:h + 1])

        es_a.close()   # h_fm released; wo_sb halves can land in its zone

        # ---- phase C: attention (head-outer) ------------------------------
        # Wo first half + obias prefetch: issued first so the 4 MiB load
        # streams while the attention loop runs (second half at phase E).
        pool_wo0 = es_wo.enter_context(tc.tile_pool(name="slotWo0", bufs=1))
        ob_p = es_wo.enter_context(tc.tile_pool(name="ob", bufs=1))
        HVD = (VDT // 2) * D
        wo_h0 = pool_wo0.tile([128, HVD], BF16, tag="wo0")
        dma_blk_sp(wo_h0[:], wo_d[0:D // 2, :])
        obias = ob_p.tile([2, D], BF16)
        dma(obias[:], obias_d[:])
        dma(masks, masks_d[:])
        dma(expb, expb_d[:])
        dma(sel, sel_d[:])

        pool_c = es_wo.enter_context(tc.tile_pool(name="slotC", bufs=1))
        agp = es_wo.enter_context(tc.tile_pool(name="agp", bufs=2))
        attg = pool_c.tile([128, VDT * OWN], BF16, tag="c")

        # loop-order permutation: spread the 4-way-exp slots (0,1) and the
        # 2-way ones (2,3) across the head loop so the ACT engine's exp load
        # stays even instead of front-loaded. Pure bookkeeping: exchange
        # buffers are position-indexed, attg stays slot-indexed.
        PI = [0, 4, 2, 5, 1, 6, 3, 7][:HL] if HL == 8 else list(range(HL))

        def assemble(g):
            for sx in range(2):
                for hh2 in range(CHUNKS[g]):
                    h2 = PI[CH_BASE[g] + hh2]
                    g0 = agp.tile([128, OWN], BF16, tag="g")
                    dma(g0[:], ago_g[g][sx, 0, hh2 * 128:(hh2 + 1) * 128, :])
                    g1 = agp.tile([128, OWN], BF16, tag="g")
                    dma(g1[:], ago_g[g][sx, 1, hh2 * 128:(hh2 + 1) * 128, :])
                    dst = attg[:, (sx * HL + h2) * OWN:
                               (sx * HL + h2 + 1) * OWN]
                    nc.vector.tensor_scalar_mul(dst, g0[:], sel[:, 0:1])
                    nc.vector.scalar_tensor_tensor(
                        dst, g1[:], sel[:, 1:2], dst,
                        op0=ALU.mult, op1=ALU.add)

        with ExitStack() as ph:
            att_p = ph.enter_context(tc.tile_pool(name="att", bufs=4))
            bcn_p = ph.enter_context(tc.tile_pool(name="bcn", bufs=2))
            den_p = ph.enter_context(tc.tile_pool(name="den", bufs=2))
            oat_p = ph.enter_context(tc.tile_pool(name="oat", bufs=2))
            ps_s = ph.enter_context(
                tc.tile_pool(name="pss", bufs=3, space="PSUM"))
            ps_a = ph.enter_context(
                tc.tile_pool(name="psa", bufs=2, space="PSUM"))
            ps_d = ph.enter_context(
                tc.tile_pool(name="psd", bufs=2, space="PSUM"))
            ps_b = ph.enter_context(
                tc.tile_pool(name="psb", bufs=1, space="PSUM"))
            for hi in range(HL):
                h = PI[hi]
                exch = exch_g[CH_OF[hi]]
                hrow = (hi - CH_BASE[CH_OF[hi]]) * 128
                for ct in range(CQ):
                    njt = min(ST, (ct + 1) * (QW // 128))
                    pav = ps_a.tile([128, QW], F32)
                    pden = ps_d.tile([1, QW], F32)
                    for jt in range(njt):
                        pss = ps_s.tile([128, QW], F32)
                        nc.tensor.matmul(
                            pss[:],
                            k_sb[:, h * S + jt * 128: h * S + (jt + 1) * 128],
                            q_sb[:, h * S + ct * QW: h * S + (ct + 1) * QW],
                            start=True, stop=True)
                        m = jt - ct * (QW // 128)
                        if 0 <= m < 4:
                            nc.vector.tensor_add(
                                pss[:], pss[:],
                                masks[:, m * 512: m * 512 + QW])
                        at = att_p.tile([128, QW], BF16)
                        G = GRAIN[h]
                        e = jt - 4 * ct
                        for u in range(QW // G):
                            if G == 128:
                                ti = 2 * (e - u) + 30
                            elif G == 256:
                                ti = 2 * e - 4 * u - 1 + 30
                            else:
                                ti = 2 * e - 3 + 30
                            col = h * NT + ti
                            nc.scalar.activation(
                                at[:, u * G:(u + 1) * G],
                                pss[:, u * G:(u + 1) * G], AF.Exp,
                                bias=expb[:, col: col + 1],
                                scale=1.0)
                        nc.tensor.matmul(
                            pav[:],
                            v_sb[:, jt * HLW + h * 128:
                                 jt * HLW + (h + 1) * 128],
                            at[:], start=(jt == 0), stop=(jt == njt - 1))
                        nc.tensor.matmul(
                            pden[:], onesc, at[:],
                            start=(jt == 0), stop=(jt == njt - 1))
                    den = den_p.tile([1, 2 * QW], F32, tag="denf")
                    dsb = den[:, 0:QW]
                    rec = den[:, QW:2 * QW]
                    nc.vector.tensor_copy(dsb, pden[:])
                    nc.vector.reciprocal(rec, dsb)
                    recb = den_p.tile([1, QW], BF16, tag="denb")
                    nc.vector.tensor_copy(recb[:], rec)
                    pbc = ps_b.tile([128, QW], F32)
                    nc.tensor.matmul(pbc[:], onesr, recb[:],
                                     start=True, stop=True)
                    bcn = bcn_p.tile([128, QW], F32)
                    nc.vector.tensor_copy(bcn[:], pbc[:])
                    oat = oat_p.tile([128, QW], BF16)
                    nc.vector.scalar_tensor_tensor(
                        oat[:], pav[:], 1.0, bcn[:],
                        op0=ALU.mult, op1=ALU.mult)
                    for half in range(2):
                        a = max(ct * QW, half * OWN)
                        bnd = min((ct + 1) * QW, (half + 1) * OWN)
                        if a < bnd:
                            dma_sp(
                                exch[half, hrow:hrow + 128,
                                     a - half * OWN: bnd - half * OWN],
                                oat[:, a - ct * QW: bnd - ct * QW])
                # chunk done -> exchange it under the remaining heads'
                # compute (only the last chunk's exchange is exposed)
                if hi == HL - 1 or CH_OF[hi + 1] != CH_OF[hi]:
                    g = CH_OF[hi]
                    nc.gpsimd.collective_compute(
                        "AllGather", ALU.bypass, replica_groups=groups,
                        ins=[exch_g[g].opt()], outs=[ago_g[g].opt()])
                    if g >= 1:
                        assemble(g - 1)
            assemble(NG - 1)

        es_qkv.close()

        # ---- phase E: out-proj + residual + LN2 + transpose --------------
        pool_wo1 = es_wo.enter_context(tc.tile_pool(name="slotWo1", bufs=1))
        wo_h1 = pool_wo1.tile([128, HVD], BF16, tag="wo1")
        dma_blk_sp(wo_h1[:], wo_d[D // 2:D, :])

        w1_p = es_b.enter_context(tc.tile_pool(name="w1", bufs=2,
                                               side="right"))
        pool_b = es_b.enter_context(tc.tile_pool(name="slotB", bufs=1, side="right"))
        h2_fm = pool_b.tile([128, DT * OWN], BF16, tag="b")
        with ExitStack() as ph:
            xo_p = ph.enter_context(tc.tile_pool(name="xo", bufs=2))
            x2_p = ph.enter_context(tc.tile_pool(name="x2", bufs=2))
            h2_p = ph.enter_context(tc.tile_pool(name="h2", bufs=1))
            stat = ph.enter_context(tc.tile_pool(name="e_stat", bufs=2))
            ps_o = ph.enter_context(
                tc.tile_pool(name="pso", bufs=2, space="PSUM"))
            tps = ph.enter_context(
                tc.tile_pool(name="etps", bufs=6, space="PSUM"))

            for it in range(OTT):
                x2 = x2_p.tile([128, D], F32)
                for dc in range(D // 512):
                    po = ps_o.tile([128, 512], F32)
                    nc.tensor.matmul(
                        po[:], ones2, obias[:, dc * 512:(dc + 1) * 512],
                        start=True, stop=False)
                    for v in range(VDT):
                        wo_sb = wo_h0 if v < VDT // 2 else wo_h1
                        vv = v if v < VDT // 2 else v - VDT // 2
                        nc.tensor.matmul(
                            po[:],
                            attg[:, v * OWN + it * 128:
                                 v * OWN + (it + 1) * 128],
                            wo_sb[:, vv * D + dc * 512: vv * D + (dc + 1) * 512],
                            start=False, stop=(v == VDT - 1))
                    xo = xo_p.tile([128, 512], F32)
                    dma_sp(
                        xo[:],
                        x_own_d[it * 128:(it + 1) * 128,
                                dc * 512:(dc + 1) * 512])
                    nc.vector.tensor_add(
                        x2[:, dc * 512:(dc + 1) * 512], po[:], xo[:])
                dma_sp(x2buf[it * 128:(it + 1) * 128, :], x2[:])
                h2 = h2_p.tile([128, D], BF16)
                layernorm_tile(stat, x2, h2)
                for dt in range(DT):
                    tp = tps.tile([128, 128], BF16)
                    nc.tensor.transpose(
                        tp[:], h2[:, dt * 128:(dt + 1) * 128], ident)
                    dst2 = h2_fm[:, dt * OWN + it * 128:
                                 dt * OWN + (it + 1) * 128]
                    if dt % 2 == 0:
                        nc.vector.tensor_copy(dst2, tp[:])
                    else:
                        nc.scalar.copy(dst2, tp[:])

        es_wo.close()

        # ---- phase F: fused MLP — GELU output stays resident in SBUF -----
        # Per 512-token half: up-proj all F into g (f-partition layout, the
        # exact lhsT layout down-proj needs), then down-proj streaming w2 in
        # 256-col chunks. No DRAM round-trip for g; w1 loaded in 256-col
        # chunks (512B lines) instead of 128-col (256B lines).
        w2_p = top.enter_context(tc.tile_pool(name="w2", bufs=2))
        g_p = top.enter_context(tc.tile_pool(name="gsb", bufs=1))
        with ExitStack() as ph:
            b2_p = ph.enter_context(tc.tile_pool(name="b2", bufs=1))
            x2s_p = ph.enter_context(tc.tile_pool(name="x2s", bufs=2))
            o_p = ph.enter_context(tc.tile_pool(name="osb", bufs=2))
            ps_m = ph.enter_context(
                tc.tile_pool(name="psm", bufs=2, space="PSUM"))
            ps_d2 = ph.enter_context(
                tc.tile_pool(name="psd2", bufs=2, space="PSUM"))
            dma(b1c, b1c_d[:])
            b2row = b2_p.tile([2, D], BF16)
            dma(b2row[:], b2row_d[:])
            HTOK = 512                      # tokens per fused half
            W1C = 512                       # w1 f-cols per load
            W2C = 256                       # w2 d-cols per load
            for hf in range(OWN // HTOK):
                g = g_p.tile([128, FT * HTOK], BF16, tag="g")
                for fc in range(F // W1C):
                    w1t = w1_p.tile([128, DT * W1C], BF16)
                    dma_blk(w1t[:], w1_d[:, fc * W1C:(fc + 1) * W1C])
                    for sub in range(W1C // 128):
                        ft = fc * (W1C // 128) + sub
                        ps = ps_m.tile([128, HTOK], F32)
                        for dt in range(DT):
                            nc.tensor.matmul(
                                ps[:],
                                w1t[:, dt * W1C + sub * 128:
                                    dt * W1C + (sub + 1) * 128],
                                h2_fm[:, dt * OWN + hf * HTOK:
                                      dt * OWN + (hf + 1) * HTOK],
                                start=(dt == 0), stop=(dt == DT - 1))
                        nc.scalar.activation(
                            g[:, ft * HTOK:(ft + 1) * HTOK], ps[:], GELU,
                            bias=b1c[:, ft:ft + 1], scale=1.0)
                for dc in range(D // W2C):
                    w2t = w2_p.tile([128, FT * W2C], BF16)
                    dma_blk(w2t[:], w2_d[:, dc * W2C:(dc + 1) * W2C])
                    for it2 in range(HTOK // 128):
                        it = hf * (HTOK // 128) + it2
                        ps = ps_d2.tile([128, W2C], F32)
                        nc.tensor.matmul(
                            ps[:], ones2, b2row[:, dc * W2C:(dc + 1) * W2C],
                            start=True, stop=False)
                        for ft in range(FT):
                            nc.tensor.matmul(
                                ps[:],
                                g[:, ft * HTOK + it2 * 128:
                                    ft * HTOK + (it2 + 1) * 128],
                                w2t[:, ft * W2C:(ft + 1) * W2C],
                                start=False, stop=(ft == FT - 1))
                        x2t = x2s_p.tile([128, W2C], F32)
                        dma_sp(x2t[:],
                            x2buf[it * 128:(it + 1) * 128,
                                  dc * W2C:(dc + 1) * W2C])
                        ot = o_p.tile([128, W2C], F32)
                        nc.vector.tensor_add(ot[:], ps[:], x2t[:])
                        dma_sp(out_d[it * 128:(it + 1) * 128,
                                     dc * W2C:(dc + 1) * W2C],
                               ot[:])

        es_b.close()

    if legalize:
        _legalize_waits(nc)
    return nc


def _legalize_waits(nc):
    """walrus on this container encodes at most ONE sync wait per DMA/branch
    instruction. Tile emits several (reader-WAR + DMA-lane WAW). Waits are
    executed by the issuing engine's sequencer in program order, so hoisting
    all-but-one wait onto wait-only EventSemaphore instructions inserted
    immediately before it on the same engine stream is semantics-preserving."""
    n_split = 0
    for fn in nc.m.functions:
        for bb in fn.blocks:
            out = []
            for inst in bb.instructions:
                si = inst.sync_info
                waits = list(si.on_wait) if si and si.on_wait else []
                if len(waits) > 1:
                    # merge same-sem waits to the max value
                    merged = {}
                    for w in waits:
                        k = (w.sync_type, w.id, w.wait_mode)
                        if k not in merged or merged[k].wait_value < w.wait_value:
                            merged[k] = w
                    waits = list(merged.values())
                    for w in waits[:-1]:
                        es = mybir.InstEventSemaphore(
                            name=f"{inst.name}-wsplit{n_split}",
                            engine=inst.engine,
                            ins=[], outs=[],
                            sync_info=mybir.SyncInfo(on_wait=[w], on_update=[]),
                        )
                        out.append(es)
                        n_split += 1
                    inst.sync_info = mybir.SyncInfo(
                        on_wait=[waits[-1]],
                        on_update=list(si.on_update) if si.on_update else [])
                out.append(inst)
            bb.instructions[:] = out


# ------------------------------------------------------------- the entry ---

_BUILT = {}


def _get_nc(cfg_key=None):
    if "nc" not in _BUILT:
        _BUILT["nc"] = build_kernel(REAL_CFG)
    return _BUILT["nc"]


def kernel(**inputs):
    cfg = REAL_CFG
    c = _cfg_derived(cfg)
    nc = _get_nc()
    in_maps = [make_core_inputs(cfg, inputs, core) for core in range(8)]
    from concourse.bass_utils import run_bass_kernel_spmd
    res = run_bass_kernel_spmd(nc, in_maps, list(range(8)))
    B = np.asarray(inputs["x"]).shape[0]
    S, D, OWN = cfg["S"], cfg["D"], c["OWN"]
    out = np.empty((B, S, D), np.float32)
    for core in range(8):
        b, r = core // 2, core % 2
        out[b, r * OWN:(r + 1) * OWN, :] = res.results[core]["out"]
    return out



# revision 6
# speedup vs baseline: 1.4121x; 1.0008x over previous
"""Trainium2 Bass kernel for a pre-LN transformer block (dense_transformer).

Reference computation (fp32, per batch element):
    x = x + Attn(LN1(x));  x = x + MLP(LN2(x))
with 16-head causal ALiBi attention (S=2048, D=2048) and a 4*D GELU MLP.

Distribution: 4 batches x 2-way head/tensor parallel = 8 cores.
Core c handles batch c//2 with pair-rank r=c%2:
  - attention: 8 local heads (r*8..r*8+7), all 2048 query positions. Scores
    are computed transposed [j(key) x i(query)]; the full ALiBi+softmax term
    rides the ACT-exp per-partition bias alone: the bias encodes
    s_h*(j - center_u) for a per-query sub-chunk center, and the implied
    per-query-column scale e^{s_h*(i-center_u)} is constant across key tiles
    so it cancels between the AV accumulation and the softmax denominator.
    No seed matmul is needed. Steep-slope head slots use 128-wide exp
    sub-chunks (fp32/bf16 range control); shallow slots use 256/512-wide
    ones (fewer ACT instructions). Head slot grain is compile-time; which
    head lives in a slot is per-core DATA, so the stream stays SPMD.
  - softmax denominators via ones-lhsT matmuls accumulated alongside AV;
    normalization is fused into the AV PSUM->SBUF copy using a K=1 broadcast
    matmul of the reciprocal row.
  - a pair AllGather (chunked by head group) swaps attention halves so each
    core owns 1024 tokens for the output projection, residual, LN2 and MLP;
    gathered chunks are assembled into the Wo operand DURING the remaining
    heads' attention compute (only the last 1-head chunk is exposed).
All per-core variation (weight slices, ALiBi slopes, token offsets) is input
DATA; the instruction stream is identical on all 8 cores (SPMD).

v3 structural changes vs v2 (same math):
  - fused MLP: GELU output g stays resident in SBUF in f-partition layout
    (exactly the lhsT layout the down-projection needs), processed in two
    512-token halves; the 48 MiB DRAM round-trip of v2 (gbuf write + narrow
    256B-line reload) is gone. w1 streams in 512-col chunks (1KB DMA lines,
    ~3x the measured single-queue bandwidth of 256B lines), w2 in 256-col
    chunks double-buffered.
  - two DMA queues: residual/x traffic, attention-output exchange writes and
    the Wo halves ride the SP engine's hardware-DGE queue; weight streams,
    collectives and AllGather-result reads keep the gpsimd software-DGE
    queue (assembly reads sit right behind their collective there, so they
    never head-of-line-block anything). This removes FIFO blocking between
    independent streams (all DMA previously serialized on one queue).
    (Do NOT issue DMAs from the ACT queue: their sync waits stall the ACT
    sequencer and delay GELU/exp work behind them - measured regression.)
  - head loop order interleaves the exp-heavy slots (PI permutation) so the
    ACT engine's exp load is spread across the attention phase instead of
    front-loaded; exchange buffers are position-indexed, attg slot-indexed.
  - x streamed in bf16 for the LN1 pass (residual path keeps fp32 x_own),
    halving the startup DMA and doubling DVE throughput there; transpose
    PSUM->SBUF copies alternate DVE/ACT.
  - scores PSUM pool depth 3 so the PE can run ahead of ACT exp.

Measured on 8xTRN2 (slope method, amortizing the ~85 ms axon round-trip and
~0.5 ms per-exec runtime overhead): v2 3.77 ms -> v3 2.43 ms per execution
(T(17)-T(1) lever arm; shorter arms read 0.1-0.2 ms lower on lucky RTTs).

The walrus in this container encodes at most ONE sync wait per instruction,
so _legalize_waits() splits every multi-wait instruction into wait-only
EventSemaphore instructions inserted immediately before it on the same
engine stream - order-preserving, so semantics are unchanged.
"""

import os
import sys

for _p in ("/opt/trn_rl_repo", "/opt/trn_rl_repo/concourse"):
    if os.path.isdir(_p) and _p not in sys.path:
        sys.path.append(_p)

import numpy as np
import ml_dtypes

import concourse.bass as bass
import concourse.mybir as mybir
import concourse.tile as tile
from contextlib import ExitStack

BF16 = mybir.dt.bfloat16
F32 = mybir.dt.float32
AF = mybir.ActivationFunctionType
ALU = mybir.AluOpType

REAL_CFG = dict(S=2048, D=2048, F=8192, H=16, GELU="Gelu")
LN_EPS = 1e-5
NEG = -1.0e6  # causal mask additive value (pre-exp)


def _cfg_derived(cfg):
    S, D, F, H = cfg["S"], cfg["D"], cfg["F"], cfg["H"]
    d = dict(cfg)
    d["HL"] = H // 2              # local heads per core
    d["HLW"] = d["HL"] * 128      # local head width (vd)
    d["DT"] = D // 128
    d["ST"] = S // 128
    d["QW"] = 512                 # q-chunk width (asserted below)
    d["CQ"] = S // 512
    d["OWN"] = S // 2
    d["OTT"] = d["OWN"] // 128
    d["FT"] = F // 128
    d["VDT"] = H
    d["HG"] = d["HL"] // 2        # heads per exchange group (2 groups)
    assert S % 512 == 0 and D % 512 == 0 and F % 512 == 0
    return d


# ------------------------------------------------------------ host prep ---


def _bf(x):
    return np.asarray(x, np.float32).astype(ml_dtypes.bfloat16)


def _split3(v):
    """Split fp32 array (last axis vectors) into 3 bf16 rows summing to it."""
    v = np.asarray(v, np.float32)
    r0 = v.astype(ml_dtypes.bfloat16)
    rem = v - r0.astype(np.float32)
    r1 = rem.astype(ml_dtypes.bfloat16)
    r2 = (rem - r1.astype(np.float32)).astype(ml_dtypes.bfloat16)
    return np.stack([r0, r1, r2])


def make_core_inputs(cfg, inputs, core):
    c = _cfg_derived(cfg)
    S, D, F, H, HL, ST = c["S"], c["D"], c["F"], c["H"], c["HL"], c["ST"]
    HLW, OWN, FT = c["HLW"], c["OWN"], c["FT"]
    b, r = core // 2, core % 2
    hd = 128
    f32 = np.float32

    x = np.asarray(inputs["x"][b], f32)
    g1 = np.asarray(inputs["ln1_w"], f32)
    c1 = np.asarray(inputs["ln1_b"], f32)
    g2 = np.asarray(inputs["ln2_w"], f32)
    c2 = np.asarray(inputs["ln2_b"], f32)
    Wqkv = np.asarray(inputs["Wqkv"], f32)
    bqkv = np.asarray(inputs["bqkv"], f32)
    Wo = np.asarray(inputs["Wo"], f32)
    bo = np.asarray(inputs["bo"], f32)
    W1 = np.asarray(inputs["W1"], f32)
    b1 = np.asarray(inputs["b1"], f32)
    W2 = np.asarray(inputs["W2"], f32)
    b2 = np.asarray(inputs["b2"], f32)
    slopes = np.asarray(inputs["slopes"], f32)

    Wq, Wk, Wv = Wqkv[:D], Wqkv[D:2 * D], Wqkv[2 * D:]
    bq, bk, bv = bqkv[:D], bqkv[D:2 * D], bqkv[2 * D:]

    lo, hi = r * HLW, (r + 1) * HLW
    sc = 1.0 / np.sqrt(hd)

    Wq_l = Wq[lo:hi] * g1[None, :]
    Wk_l = Wk[lo:hi] * g1[None, :]
    Wv_l = Wv[lo:hi] * g1[None, :]
    qb = (Wq[lo:hi] @ c1 + bq[lo:hi]) * sc
    kb = Wk[lo:hi] @ c1 + bk[lo:hi]
    cbo = Wo @ (Wv @ c1 + bv) + bo          # v-bias + bo folded constant [D]

    W1p = W1 * g2[None, :]
    b1p = W1 @ c2 + b1

    heads = np.arange(r * HL, r * HL + HL)
    sl = slopes[heads]
    jpos = np.arange(S, dtype=f32)

    # exp bias table, half-step t grid: col (h, ti) = s_h*(j + 128*t - 64)
    # with t = -15 + 0.5*ti. The implied per-query-column scale
    # e^{s_h*(i-center)} is constant across key tiles for a fixed query
    # column, so it cancels between the AV accumulation and the denominator.
    # Steep-slope slots use 128-wide exp sub-chunks (range control); shallow
    # slots use 256/512-wide ones (fewer ACT instructions).
    NT = 37
    expbias = np.zeros((128, HL * NT), f32)
    for h in range(HL):
        for ti in range(NT):
            expbias[:, h * NT + ti] = sl[h] * (
                jpos[:128] + 128.0 * (-15.0 + 0.5 * ti) - 64.0)

    masks = np.zeros((128, 4 * 512), f32)
    jj = np.arange(128)[:, None]
    ii = np.arange(512)[None, :]
    for m in range(4):
        masks[:, m * 512:(m + 1) * 512] = np.where(m * 128 + jj <= ii, 0.0, NEG)

    return {
        "x_full": np.ascontiguousarray(_bf(x)),
        "x_own": np.ascontiguousarray(x[r * OWN:(r + 1) * OWN]),
        "wq_t": np.ascontiguousarray(_bf(Wq_l.T * sc)),
        "wk_t": np.ascontiguousarray(_bf(Wk_l.T)),
        "wv_t": np.ascontiguousarray(_bf(Wv_l.T)),
        "wo_t": np.ascontiguousarray(_bf(Wo.T)),
        "w1_t": np.ascontiguousarray(_bf(W1p.T)),
        "w2_t": np.ascontiguousarray(_bf(W2.T)),
        "qb": np.ascontiguousarray(qb.reshape(HL, 128).T),
        "kb": np.ascontiguousarray(kb.reshape(HL, 128).T),
        "b1c": np.ascontiguousarray(b1p.reshape(FT, 128).T),
        "obias": np.ascontiguousarray(_split3(cbo)[:2]),
        "b2row": np.ascontiguousarray(_split3(b2)[:2]),
        "expbias": expbias,
        "masks": np.ascontiguousarray(masks.astype(ml_dtypes.bfloat16)),
        "ident": np.eye(128, dtype=f32).astype(ml_dtypes.bfloat16),
        "sel": np.ascontiguousarray(
            np.repeat((np.arange(2) == r).astype(f32)[None, :], 128, axis=0)),
    }


# ------------------------------------------------------------ the kernel ---


def build_kernel(cfg, legalize=True):
    c = _cfg_derived(cfg)
    S, D, F = c["S"], c["D"], c["F"]
    HL, HLW, DT, ST = c["HL"], c["HLW"], c["DT"], c["ST"]
    CQ, QW, OWN, OTT, FT, VDT, HG = (c["CQ"], c["QW"], c["OWN"], c["OTT"],
                                     c["FT"], c["VDT"], c["HG"])
    GELU = getattr(AF, cfg["GELU"])

    # Single SWDGE sem lane: every DMA rides one FIFO queue (qPoolDynamic)
    # anyway, but Tile's default 8-lane round-robin sem assignment makes
    # slot-reuse DMAs wait on several DMASW sems at once, and the DMA ISA
    # encodes at most 2 waits (walrus "Too many sync wait commands").
    import concourse.tile_sem_assignment as tsa
    tsa.NUM_SWDGE_GLOBAL_SEMS = 1

    nc = bass.Bass()

    def param(name, shape, dt):
        return nc.declare_dram_parameter(name, shape, dt, isOutput=False)

    x_full_d = param("x_full", [S, D], BF16)
    x_own_d = param("x_own", [OWN, D], F32)
    wq_d = param("wq_t", [D, HLW], BF16)
    wk_d = param("wk_t", [D, HLW], BF16)
    wv_d = param("wv_t", [D, HLW], BF16)
    wo_d = param("wo_t", [D, D], BF16)
    w1_d = param("w1_t", [D, F], BF16)
    w2_d = param("w2_t", [F, D], BF16)
    qb_d = param("qb", [128, HL], F32)
    kb_d = param("kb", [128, HL], F32)
    b1c_d = param("b1c", [128, FT], F32)
    obias_d = param("obias", [2, D], BF16)
    b2row_d = param("b2row", [2, D], BF16)
    NT = 37
    expb_d = param("expbias", [128, HL * NT], F32)
    # exp sub-chunk width per head slot (slot h pairs global heads h and h+8;
    # the steeper slope 2^-(h+1)/2 of the pair bounds the exp argument range)
    GRAIN = [128, 256, 256, 512, 512, 512, 512, 512][:HL]
    masks_d = param("masks", [128, 4 * 512], BF16)
    ident_d = param("ident", [128, 128], BF16)
    sel_d = param("sel", [128, 2], F32)
    out_d = nc.declare_dram_parameter("out", [OWN, D], F32, isOutput=True)

    groups = [[0, 1], [2, 3], [4, 5], [6, 7]]
    # exchange chunk sizes (heads): small tail chunks so the last exchange
    # (the only exposed one) is cheap
    CHUNKS = [2, 2, 2, 1, 1] if HL == 8 else [1] * HL
    NG = len(CHUNKS)
    CH_OF = []                    # head -> chunk index
    for gi, n in enumerate(CHUNKS):
        CH_OF += [gi] * n
    CH_BASE = [sum(CHUNKS[:gi]) for gi in range(NG)]

    with tile.TileContext(nc) as tc, ExitStack() as top:
        def dma(out_ap, in_ap):
            nc.gpsimd.dma_start(out_ap, in_ap)

        def dma_sp(out_ap, in_ap):
            nc.sync.dma_start(out_ap, in_ap)

        def dma_blk_sp(sbuf_ap, dram_ap):
            rows = dram_ap.shape[0]
            t = rows // 128
            nc.sync.dma_start(
                sbuf_ap.rearrange("p (t f) -> p t f", t=t),
                dram_ap.rearrange("(t p) f -> p t f", p=128))

        def dma_blk_act(sbuf_ap, dram_ap):
            rows = dram_ap.shape[0]
            t = rows // 128
            nc.scalar.dma_start(
                sbuf_ap.rearrange("p (t f) -> p t f", t=t),
                dram_ap.rearrange("(t p) f -> p t f", p=128))

        def dma_blk(sbuf_ap, dram_ap):
            """DMA DRAM [T*128, N] into SBUF [128, T*N] (block t at t*N)."""
            rows = dram_ap.shape[0]
            t = rows // 128
            dma(sbuf_ap.rearrange("p (t f) -> p t f", t=t),
                dram_ap.rearrange("(t p) f -> p t f", p=128))

        dram = top.enter_context(tc.tile_pool(name="dram", bufs=1,
                                              space="DRAM"))
        exch_g = [dram.tile([2, CHUNKS[i] * 128, OWN], BF16,
                            name=f"exch{i}", tag=f"exch{i}")
                  for i in range(NG)]
        ago_g = [dram.tile([2, 2, CHUNKS[i] * 128, OWN], BF16,
                           name=f"ago{i}", tag=f"ago{i}") for i in range(NG - 1)]
        # last group (1 head): exchanged per token-half so half0 overlaps the
        # final head's remaining compute; only half1's exchange is exposed
        agoh = [dram.tile([2, CHUNKS[NG - 1] * 128, OWN], BF16,
                          name=f"agoh{i}", tag=f"agoh{i}") for i in range(2)]
        x2buf = dram.tile([OWN, D], F32)

        es_a, es_qkv, es_wo, es_b = (ExitStack(), ExitStack(), ExitStack(),
                                     ExitStack())
        const = top.enter_context(tc.tile_pool(name="const", bufs=1))
        pool_a = es_a.enter_context(tc.tile_pool(name="slotA", bufs=1))

        # Packed constants: every tile pads to 4 KiB/partition, so the many
        # small tiles are packed into two wide ones and sliced by view.
        # bf16 pack: masks[0:2048] | ident[2048:2176] | ones[2176:2304]
        pk_b = const.tile([128, 4 * 512 + 128 + 128], BF16)
        masks = pk_b[:, 0:4 * 512]
        ident = pk_b[:, 4 * 512:4 * 512 + 128]
        _ones = pk_b[:, 4 * 512 + 128:4 * 512 + 256]
        ones3 = _ones[0:3, 0:128]
        ones2 = _ones[0:2, 0:128]
        onesc = _ones[:, 0:1]
        onesr = _ones[0:1, 0:128]
        # f32 pack: expb[0:HL*NT] | qb | kb | b1c | sel | epsc
        _c0 = HL * NT
        pk_f = const.tile([128, _c0 + 2 * HL + FT + 3], F32)
        expb = pk_f[:, 0:_c0]
        qb = pk_f[:, _c0:_c0 + HL]
        kb = pk_f[:, _c0 + HL:_c0 + 2 * HL]
        b1c = pk_f[:, _c0 + 2 * HL:_c0 + 2 * HL + FT]
        sel = pk_f[:, _c0 + 2 * HL + FT:_c0 + 2 * HL + FT + 2]
        epsc = pk_f[:, _c0 + 2 * HL + FT + 2:_c0 + 2 * HL + FT + 3]

        dma(ident, ident_d[:])
        nc.vector.memset(_ones, 1.0)
        nc.vector.memset(epsc, LN_EPS)
        dma(qb, qb_d[:])
        dma(kb, kb_d[:])

        # ---- LayerNorm helper (normalized output only; w/b pre-folded) ---
        LNG = D // 512 if D >= 512 else 1

        def layernorm_tile(stat, xt, out_bf):
            st = stat.tile([128, 6 * LNG + 4], F32, tag="lnstat")
            st6 = st[:, 0:6 * LNG]
            ag = st[:, 6 * LNG:6 * LNG + 2]
            sd = st[:, 6 * LNG + 2:6 * LNG + 3]
            rr = st[:, 6 * LNG + 3:6 * LNG + 4]
            for g in range(LNG):
                nc.vector.bn_stats(st6[:, 6 * g:6 * (g + 1)],
                                   xt[:, 512 * g:512 * (g + 1)])
            nc.vector.bn_aggr(ag, st6)
            nc.scalar.activation(sd, ag[:, 1:2], AF.Sqrt,
                                 bias=epsc, scale=1.0)
            nc.vector.reciprocal(rr, sd)
            nc.vector.tensor_scalar(
                out_bf[:], xt[:], scalar1=ag[:, 0:1], scalar2=rr,
                op0=ALU.subtract, op1=ALU.mult)

        # ---- phase A: LN1 + transpose into h_fm ---------------------------
        h_fm = pool_a.tile([128, DT * S], BF16, tag="a")
        with ExitStack() as ph:
            xpool = ph.enter_context(tc.tile_pool(name="ln_x", bufs=3))
            stat = ph.enter_context(tc.tile_pool(name="ln_stat", bufs=3))
            hbf = ph.enter_context(tc.tile_pool(name="ln_h", bufs=3))
            tps = ph.enter_context(
                tc.tile_pool(name="tps", bufs=6, space="PSUM"))
            for tt in range(ST):
                xt = xpool.tile([128, D], BF16)
                dma_sp(xt[:], x_full_d[tt * 128:(tt + 1) * 128, :])
                ht = hbf.tile([128, D], BF16)
                layernorm_tile(stat, xt, ht)
                for dt in range(DT):
                    tp = tps.tile([128, 128], BF16)
                    nc.tensor.transpose(
                        tp[:], ht[:, dt * 128:(dt + 1) * 128], ident)
                    dst = h_fm[:, dt * S + tt * 128: dt * S + (tt + 1) * 128]
                    if dt % 2 == 0:
                        nc.vector.tensor_copy(dst, tp[:])
                    else:
                        nc.scalar.copy(dst, tp[:])

        # ---- phase B: Q, K and V projections ------------------------------
        pool_q = es_qkv.enter_context(tc.tile_pool(name="slotQ", bufs=1, side="right"))
        pool_k = es_qkv.enter_context(tc.tile_pool(name="slotK", bufs=1, side="right"))
        pool_v = es_qkv.enter_context(tc.tile_pool(name="slotV", bufs=1, side="right"))
        q_sb = pool_q.tile([128, HL * S], BF16, tag="q")
        k_sb = pool_k.tile([128, HL * S], BF16, tag="k")
        v_sb = pool_v.tile([128, ST * HLW], BF16, tag="v")
        with ExitStack() as ph:
            mps = ph.enter_context(
                tc.tile_pool(name="bps", bufs=4, space="PSUM"))
            wpool = ph.enter_context(tc.tile_pool(name="slotW", bufs=1, side="right"))

            wv_sb = wpool.tile([128, DT * HLW], BF16, tag="w")
            dma_blk(wv_sb[:], wv_d.ap())
            VCW = min(512, HLW)
            for jt in range(ST):
                for vc in range(HLW // VCW):
                    ps = mps.tile([128, VCW], F32)
                    for dt in range(DT):
                        nc.tensor.matmul(
                            ps[:],
                            h_fm[:, dt * S + jt * 128: dt * S + (jt + 1) * 128],
                            wv_sb[:, dt * HLW + vc * VCW:
                                  dt * HLW + (vc + 1) * VCW],
                            start=(dt == 0), stop=(dt == DT - 1))
                    nc.vector.tensor_copy(
                        v_sb[:, jt * HLW + vc * VCW:
                             jt * HLW + (vc + 1) * VCW],
                        ps[:])

            wk_sb = wpool.tile([128, DT * HLW], BF16, tag="w")
            dma_blk(wk_sb[:], wk_d.ap())
            for h in range(HL):
                for ch in range(CQ):
                    ps = mps.tile([128, QW], F32)
                    for dt in range(DT):
                        nc.tensor.matmul(
                            ps[:],
                            wk_sb[:, dt * HLW + h * 128:
                                  dt * HLW + (h + 1) * 128],
                            h_fm[:, dt * S + ch * QW: dt * S + (ch + 1) * QW],
                            start=(dt == 0), stop=(dt == DT - 1))
                    nc.vector.tensor_scalar_add(
                        k_sb[:, h * S + ch * QW: h * S + (ch + 1) * QW],
                        ps[:], kb[:, h:h + 1])

            wq_sb = wpool.tile([128, DT * HLW], BF16, tag="w")
            dma_blk(wq_sb[:], wq_d.ap())
            for h in range(HL):
                for ch in range(CQ):
                    ps = mps.tile([128, QW], F32)
                    for dt in range(DT):
                        nc.tensor.matmul(
                            ps[:],
                            wq_sb[:, dt * HLW + h * 128:
                                  dt * HLW + (h + 1) * 128],
                            h_fm[:, dt * S + ch * QW: dt * S + (ch + 1) * QW],
                            start=(dt == 0), stop=(dt == DT - 1))
                    nc.vector.tensor_scalar_add(
                        q_sb[:, h * S + ch * QW: h * S + (ch + 1) * QW],
                        ps[:], qb[:, h:h + 1])

        es_a.close()   # h_fm released; wo_sb halves can land in its zone

        # ---- phase C: attention (head-outer) ------------------------------
        # Wo first half + obias prefetch: issued first so the 4 MiB load
        # streams while the attention loop runs (second half at phase E).
        pool_wo0 = es_wo.enter_context(tc.tile_pool(name="slotWo0", bufs=1))
        ob_p = es_wo.enter_context(tc.tile_pool(name="ob", bufs=1))
        HVD = (VDT // 2) * D
        wo_h0 = pool_wo0.tile([128, HVD], BF16, tag="wo0")
        dma_blk_sp(wo_h0[:], wo_d[0:D // 2, :])
        obias = ob_p.tile([2, D], BF16)
        dma(obias[:], obias_d[:])
        dma(masks, masks_d[:])
        dma(expb, expb_d[:])
        dma(sel, sel_d[:])

        pool_c = es_wo.enter_context(tc.tile_pool(name="slotC", bufs=1))
        agp = es_wo.enter_context(tc.tile_pool(name="agp", bufs=2))
        attg = pool_c.tile([128, VDT * OWN], BF16, tag="c")

        # loop-order permutation: spread the 4-way-exp slots (0,1) and the
        # 2-way ones (2,3) across the head loop so the ACT engine's exp load
        # stays even instead of front-loaded. Pure bookkeeping: exchange
        # buffers are position-indexed, attg stays slot-indexed.
        PI = [0, 4, 2, 5, 1, 6, 3, 7][:HL] if HL == 8 else list(range(HL))

        def assemble(g):
            for sx in range(2):
                for hh2 in range(CHUNKS[g]):
                    h2 = PI[CH_BASE[g] + hh2]
                    g0 = agp.tile([128, OWN], BF16, tag="g")
                    dma(g0[:], ago_g[g][sx, 0, hh2 * 128:(hh2 + 1) * 128, :])
                    g1 = agp.tile([128, OWN], BF16, tag="g")
                    dma(g1[:], ago_g[g][sx, 1, hh2 * 128:(hh2 + 1) * 128, :])
                    dst = attg[:, (sx * HL + h2) * OWN:
                               (sx * HL + h2 + 1) * OWN]
                    nc.vector.tensor_scalar_mul(dst, g0[:], sel[:, 0:1])
                    nc.vector.scalar_tensor_tensor(
                        dst, g1[:], sel[:, 1:2], dst,
                        op0=ALU.mult, op1=ALU.add)

        with ExitStack() as ph:
            att_p = ph.enter_context(tc.tile_pool(name="att", bufs=4))
            bcn_p = ph.enter_context(tc.tile_pool(name="bcn", bufs=2))
            den_p = ph.enter_context(tc.tile_pool(name="den", bufs=2))
            oat_p = ph.enter_context(tc.tile_pool(name="oat", bufs=2))
            ps_s = ph.enter_context(
                tc.tile_pool(name="pss", bufs=3, space="PSUM"))
            ps_a = ph.enter_context(
                tc.tile_pool(name="psa", bufs=2, space="PSUM"))
            ps_d = ph.enter_context(
                tc.tile_pool(name="psd", bufs=2, space="PSUM"))
            ps_b = ph.enter_context(
                tc.tile_pool(name="psb", bufs=1, space="PSUM"))
            for hi in range(HL):
                h = PI[hi]
                exch = exch_g[CH_OF[hi]]
                hrow = (hi - CH_BASE[CH_OF[hi]]) * 128
                for ct in range(CQ):
                    njt = min(ST, (ct + 1) * (QW // 128))
                    pav = ps_a.tile([128, QW], F32)
                    pden = ps_d.tile([1, QW], F32)
                    for jt in range(njt):
                        pss = ps_s.tile([128, QW], F32)
                        nc.tensor.matmul(
                            pss[:],
                            k_sb[:, h * S + jt * 128: h * S + (jt + 1) * 128],
                            q_sb[:, h * S + ct * QW: h * S + (ct + 1) * QW],
                            start=True, stop=True)
                        m = jt - ct * (QW // 128)
                        if 0 <= m < 4:
                            nc.vector.tensor_add(
                                pss[:], pss[:],
                                masks[:, m * 512: m * 512 + QW])
                        at = att_p.tile([128, QW], BF16)
                        G = GRAIN[h]
                        e = jt - 4 * ct
                        for u in range(QW // G):
                            if G == 128:
                                ti = 2 * (e - u) + 30
                            elif G == 256:
                                ti = 2 * e - 4 * u - 1 + 30
                            else:
                                ti = 2 * e - 3 + 30
                            col = h * NT + ti
                            nc.scalar.activation(
                                at[:, u * G:(u + 1) * G],
                                pss[:, u * G:(u + 1) * G], AF.Exp,
                                bias=expb[:, col: col + 1],
                                scale=1.0)
                        nc.tensor.matmul(
                            pav[:],
                            v_sb[:, jt * HLW + h * 128:
                                 jt * HLW + (h + 1) * 128],
                            at[:], start=(jt == 0), stop=(jt == njt - 1))
                        nc.tensor.matmul(
                            pden[:], onesc, at[:],
                            start=(jt == 0), stop=(jt == njt - 1))
                    den = den_p.tile([1, 2 * QW], F32, tag="denf")
                    dsb = den[:, 0:QW]
                    rec = den[:, QW:2 * QW]
                    nc.vector.tensor_copy(dsb, pden[:])
                    nc.vector.reciprocal(rec, dsb)
                    recb = den_p.tile([1, QW], BF16, tag="denb")
                    nc.vector.tensor_copy(recb[:], rec)
                    pbc = ps_b.tile([128, QW], F32)
                    nc.tensor.matmul(pbc[:], onesr, recb[:],
                                     start=True, stop=True)
                    bcn = bcn_p.tile([128, QW], F32)
                    nc.vector.tensor_copy(bcn[:], pbc[:])
                    oat = oat_p.tile([128, QW], BF16)
                    nc.vector.scalar_tensor_tensor(
                        oat[:], pav[:], 1.0, bcn[:],
                        op0=ALU.mult, op1=ALU.mult)
                    for half in range(2):
                        a = max(ct * QW, half * OWN)
                        bnd = min((ct + 1) * QW, (half + 1) * OWN)
                        if a < bnd:
                            dma_sp(
                                exch[half, hrow:hrow + 128,
                                     a - half * OWN: bnd - half * OWN],
                                oat[:, a - ct * QW: bnd - ct * QW])
                    if hi == HL - 1 and ct == 1:
                        nc.gpsimd.collective_compute(
                            "AllGather", ALU.bypass, replica_groups=groups,
                            ins=[exch_g[NG - 1][0:1].opt()],
                            outs=[agoh[0].opt()])
                # chunk done -> exchange it under the remaining heads'
                # compute (only the last chunk's exchange is exposed)
                if hi == HL - 1:
                    nc.gpsimd.collective_compute(
                        "AllGather", ALU.bypass, replica_groups=groups,
                        ins=[exch_g[NG - 1][1:2].opt()],
                        outs=[agoh[1].opt()])
                    assemble(NG - 2)
                elif CH_OF[hi + 1] != CH_OF[hi]:
                    g = CH_OF[hi]
                    nc.gpsimd.collective_compute(
                        "AllGather", ALU.bypass, replica_groups=groups,
                        ins=[exch_g[g].opt()], outs=[ago_g[g].opt()])
                    if g >= 1:
                        assemble(g - 1)
            h2 = PI[CH_BASE[NG - 1]]
            for sx in range(2):
                g0 = agp.tile([128, OWN], BF16, tag="g")
                dma(g0[:], agoh[0][sx, 0:128, :])
                g1 = agp.tile([128, OWN], BF16, tag="g")
                dma(g1[:], agoh[1][sx, 0:128, :])
                dst = attg[:, (sx * HL + h2) * OWN:
                           (sx * HL + h2 + 1) * OWN]
                nc.vector.tensor_scalar_mul(dst, g0[:], sel[:, 0:1])
                nc.vector.scalar_tensor_tensor(
                    dst, g1[:], sel[:, 1:2], dst,
                    op0=ALU.mult, op1=ALU.add)

        es_qkv.close()

        # ---- phase E: out-proj + residual + LN2 + transpose --------------
        pool_wo1 = es_wo.enter_context(tc.tile_pool(name="slotWo1", bufs=1))
        wo_h1 = pool_wo1.tile([128, HVD], BF16, tag="wo1")
        dma_blk_sp(wo_h1[:], wo_d[D // 2:D, :])

        w1_p = es_b.enter_context(tc.tile_pool(name="w1", bufs=2,
                                               side="right"))
        pool_b = es_b.enter_context(tc.tile_pool(name="slotB", bufs=1, side="right"))
        h2_fm = pool_b.tile([128, DT * OWN], BF16, tag="b")
        with ExitStack() as ph:
            xo_p = ph.enter_context(tc.tile_pool(name="xo", bufs=2))
            x2_p = ph.enter_context(tc.tile_pool(name="x2", bufs=2))
            h2_p = ph.enter_context(tc.tile_pool(name="h2", bufs=1))
            stat = ph.enter_context(tc.tile_pool(name="e_stat", bufs=2))
            ps_o = ph.enter_context(
                tc.tile_pool(name="pso", bufs=2, space="PSUM"))
            tps = ph.enter_context(
                tc.tile_pool(name="etps", bufs=6, space="PSUM"))

            for it in range(OTT):
                x2 = x2_p.tile([128, D], F32)
                for dc in range(D // 512):
                    po = ps_o.tile([128, 512], F32)
                    nc.tensor.matmul(
                        po[:], ones2, obias[:, dc * 512:(dc + 1) * 512],
                        start=True, stop=False)
                    for v in range(VDT):
                        wo_sb = wo_h0 if v < VDT // 2 else wo_h1
                        vv = v if v < VDT // 2 else v - VDT // 2
                        nc.tensor.matmul(
                            po[:],
                            attg[:, v * OWN + it * 128:
                                 v * OWN + (it + 1) * 128],
                            wo_sb[:, vv * D + dc * 512: vv * D + (dc + 1) * 512],
                            start=False, stop=(v == VDT - 1))
                    xo = xo_p.tile([128, 512], F32)
                    dma_sp(
                        xo[:],
                        x_own_d[it * 128:(it + 1) * 128,
                                dc * 512:(dc + 1) * 512])
                    nc.vector.tensor_add(
                        x2[:, dc * 512:(dc + 1) * 512], po[:], xo[:])
                dma_sp(x2buf[it * 128:(it + 1) * 128, :], x2[:])
                h2 = h2_p.tile([128, D], BF16)
                layernorm_tile(stat, x2, h2)
                for dt in range(DT):
                    tp = tps.tile([128, 128], BF16)
                    nc.tensor.transpose(
                        tp[:], h2[:, dt * 128:(dt + 1) * 128], ident)
                    dst2 = h2_fm[:, dt * OWN + it * 128:
                                 dt * OWN + (it + 1) * 128]
                    if dt % 2 == 0:
                        nc.vector.tensor_copy(dst2, tp[:])
                    else:
                        nc.scalar.copy(dst2, tp[:])

        es_wo.close()

        # ---- phase F: fused MLP — GELU output stays resident in SBUF -----
        # Per 512-token half: up-proj all F into g (f-partition layout, the
        # exact lhsT layout down-proj needs), then down-proj streaming w2 in
        # 256-col chunks. No DRAM round-trip for g; w1 loaded in 256-col
        # chunks (512B lines) instead of 128-col (256B lines).
        w2_p = top.enter_context(tc.tile_pool(name="w2", bufs=2))
        g_p = top.enter_context(tc.tile_pool(name="gsb", bufs=1))
        with ExitStack() as ph:
            b2_p = ph.enter_context(tc.tile_pool(name="b2", bufs=1))
            x2s_p = ph.enter_context(tc.tile_pool(name="x2s", bufs=2))
            o_p = ph.enter_context(tc.tile_pool(name="osb", bufs=2))
            ps_m = ph.enter_context(
                tc.tile_pool(name="psm", bufs=2, space="PSUM"))
            ps_d2 = ph.enter_context(
                tc.tile_pool(name="psd2", bufs=2, space="PSUM"))
            dma(b1c, b1c_d[:])
            b2row = b2_p.tile([2, D], BF16)
            dma(b2row[:], b2row_d[:])
            HTOK = 512                      # tokens per fused half
            W1C = 512                       # w1 f-cols per load
            W2C = 256                       # w2 d-cols per load
            for hf in range(OWN // HTOK):
                g = g_p.tile([128, FT * HTOK], BF16, tag="g")
                for fc in range(F // W1C):
                    w1t = w1_p.tile([128, DT * W1C], BF16)
                    dma_blk(w1t[:], w1_d[:, fc * W1C:(fc + 1) * W1C])
                    for sub in range(W1C // 128):
                        ft = fc * (W1C // 128) + sub
                        ps = ps_m.tile([128, HTOK], F32)
                        for dt in range(DT):
                            nc.tensor.matmul(
                                ps[:],
                                w1t[:, dt * W1C + sub * 128:
                                    dt * W1C + (sub + 1) * 128],
                                h2_fm[:, dt * OWN + hf * HTOK:
                                      dt * OWN + (hf + 1) * HTOK],
                                start=(dt == 0), stop=(dt == DT - 1))
                        nc.scalar.activation(
                            g[:, ft * HTOK:(ft + 1) * HTOK], ps[:], GELU,
                            bias=b1c[:, ft:ft + 1], scale=1.0)
                for dc in range(D // W2C):
                    w2t = w2_p.tile([128, FT * W2C], BF16)
                    dma_blk(w2t[:], w2_d[:, dc * W2C:(dc + 1) * W2C])
                    for it2 in range(HTOK // 128):
                        it = hf * (HTOK // 128) + it2
                        ps = ps_d2.tile([128, W2C], F32)
                        nc.tensor.matmul(
                            ps[:], ones2, b2row[:, dc * W2C:(dc + 1) * W2C],
                            start=True, stop=False)
                        for ft in range(FT):
                            nc.tensor.matmul(
                                ps[:],
                                g[:, ft * HTOK + it2 * 128:
                                    ft * HTOK + (it2 + 1) * 128],
                                w2t[:, ft * W2C:(ft + 1) * W2C],
                                start=False, stop=(ft == FT - 1))
                        x2t = x2s_p.tile([128, W2C], F32)
                        dma_sp(x2t[:],
                            x2buf[it * 128:(it + 1) * 128,
                                  dc * W2C:(dc + 1) * W2C])
                        ot = o_p.tile([128, W2C], F32)
                        nc.vector.tensor_add(ot[:], ps[:], x2t[:])
                        dma_sp(out_d[it * 128:(it + 1) * 128,
                                     dc * W2C:(dc + 1) * W2C],
                               ot[:])

        es_b.close()

    if legalize:
        _legalize_waits(nc)
    return nc


def _legalize_waits(nc):
    """walrus on this container encodes at most ONE sync wait per DMA/branch
    instruction. Tile emits several (reader-WAR + DMA-lane WAW). Waits are
    executed by the issuing engine's sequencer in program order, so hoisting
    all-but-one wait onto wait-only EventSemaphore instructions inserted
    immediately before it on the same engine stream is semantics-preserving."""
    n_split = 0
    for fn in nc.m.functions:
        for bb in fn.blocks:
            out = []
            for inst in bb.instructions:
                si = inst.sync_info
                waits = list(si.on_wait) if si and si.on_wait else []
                if len(waits) > 1:
                    # merge same-sem waits to the max value
                    merged = {}
                    for w in waits:
                        k = (w.sync_type, w.id, w.wait_mode)
                        if k not in merged or merged[k].wait_value < w.wait_value:
                            merged[k] = w
                    waits = list(merged.values())
                    for w in waits[:-1]:
                        es = mybir.InstEventSemaphore(
                            name=f"{inst.name}-wsplit{n_split}",
                            engine=inst.engine,
                            ins=[], outs=[],
                            sync_info=mybir.SyncInfo(on_wait=[w], on_update=[]),
                        )
                        out.append(es)
                        n_split += 1
                    inst.sync_info = mybir.SyncInfo(
                        on_wait=[waits[-1]],
                        on_update=list(si.on_update) if si.on_update else [])
                out.append(inst)
            bb.instructions[:] = out


# ------------------------------------------------------------- the entry ---

_BUILT = {}


def _get_nc(cfg_key=None):
    if "nc" not in _BUILT:
        _BUILT["nc"] = build_kernel(REAL_CFG)
    return _BUILT["nc"]


def kernel(**inputs):
    cfg = REAL_CFG
    c = _cfg_derived(cfg)
    nc = _get_nc()
    in_maps = [make_core_inputs(cfg, inputs, core) for core in range(8)]
    from concourse.bass_utils import run_bass_kernel_spmd
    res = run_bass_kernel_spmd(nc, in_maps, list(range(8)))
    B = np.asarray(inputs["x"]).shape[0]
    S, D, OWN = cfg["S"], cfg["D"], c["OWN"]
    out = np.empty((B, S, D), np.float32)
    for core in range(8):
        b, r = core // 2, core % 2
        out[b, r * OWN:(r + 1) * OWN, :] = res.results[core]["out"]
    return out

